# revision 62
# baseline (speedup 1.0000x reference)
"""Trainium2 Bass kernel for nn_AttentionFusion (dense transformer block).

Data-parallel over batch: B=8192 rows sharded as 1024 rows per NeuronCore
across 8 cores; weights replicated. On-chip layout is feature-major:
activations are stored as [128 partitions(features), k_tiles, 1024 rows],
so every matmul is out.T[m,n] = sum_k W.T[k,m] * act.T[k,n] with natural
(host-pre-transposed) weight loads and the contraction on the partition dim.

Algebraic simplifications (validated against the reference to 2e-6):
  - Cross-attention has seq len 1 -> softmax == 1 -> out = v @ wo.T + bo;
    additionally (v @ wv.T) @ wo.T = v @ (wo@wv).T is merged on the host.
  - Self-attention has seq len 2 -> softmax([a,b]) = [sig(a-b), 1-sig(a-b)].
  - LayerNorm / attention-score reductions over features (= partitions) are
    done with small matmuls against ones/head-mask matrices.

fp8 (e4m3) DoubleRow matmuls (2x contraction per instruction):
  - SA q/k/v, SA out-proj, FFN w1/w2 run as fp8 DoubleRow.
  - Weights host-quantized at 8x scale (12x for the second FFN position to
    decorrelate quantization noise between the two pooled positions);
    activations stored as value/8 in fp8 so PSUM results are at true scale.
  - Cross-attention / gate / input projections stay bf16 (noise budget).

Scheduling (655us -> 610us measured): every matmul streams 512 columns
(~216 ns) regardless of dtype (DoubleRow's 2x is the doubled contraction
per instruction), so the wins are (a) keeping the in-order PE queue free
of instructions that wait on DVE/ACT, and (b) never closing a PSUM pool
mid-kernel:
  - weights host-pre-tiled [P, MT, KT, 128]: each weight-tile DMA is one
    contiguous per-partition run (no 128B-chunk gathers).
  - ONE PSUM pool scope spans P23..P67 (a scope boundary barriers the next
    phase's first matmul on the previous phase's PSUM retirement).
  - P23: x0/x1 GEMMs share one wvo load per mt; DVE evicts (bias+residual);
    x^2 staged into the free S5/S6 slots; stat MMs emitted after the GEMMs
    (interleaved so lnp ring 2 suffices); the independent gate GEMMs (P3g,
    sigmoid fused into the ACT eviction) cover the LN chains on DVE.
  - LN fp8 copies derived from the bf16 value on ACT (out/8); for LN3 the
    gain/bias are folded into w1/b1 host-side so the FFN consumes the raw
    normalized value (gain applied later, in place, off the critical path).
  - P4 software-pipelined by one mt: the score chain (evict->mul->dmm->
    sigmoid->ab) of mt runs under mt+1's GEMM stream.
  - P5: residual-add folded into PSUM via an identity matmul, ACT evicts;
    y0's whole pipeline (GEMMs->stats->norm) runs before y1's GEMMs so
    r0_f8 is ready when the FFN starts (which begins ch0-only);
    LN3-y1 and the imgp/txtp reload hide under FFN pos0.
  - output chain: r bf16 copies stored pre-halved and the gate term
    pre-added into pooled, so each pos1 eviction is 2 DVE ops + DMA.
"""

import numpy as np
import ml_dtypes

import concourse.bacc as bacc
import concourse.mybir as mybir
import concourse.tile as tile
from concourse.bass_utils import run_bass_kernel_spmd

AF = mybir.ActivationFunctionType
ALU = mybir.AluOpType
BF16 = mybir.dt.bfloat16
F32 = mybir.dt.float32
FP8 = mybir.dt.float8e4
DR = mybir.MatmulPerfMode.DoubleRow

N_CORES = 8
B, IMG_D, TXT_D, H, NH = 8192, 1280, 2048, 1024, 16
HD = H // NH  # 64 head dim
R = B // N_CORES  # 1024 rows per core
P = 128
CH = 2  # row chunks per core
CHS = R // CH  # 512 rows per chunk
KT_I, KT_T, KT_H, KT_F = IMG_D // P, TXT_D // P, H // P, 4 * H // P
EPS = 1e-5
SA = 8.0    # fp8 weight scale, position 0 / shared
SB = 12.0   # fp8 weight scale, position 1 (FFN dual-quantization)

np_bf16 = ml_dtypes.bfloat16
np_fp8 = ml_dtypes.float8_e4m3

def _chsl(ch):
    return slice(ch * CHS, (ch + 1) * CHS)


def build():
    nc = bacc.Bacc(None, target_bir_lowering=False)

    def din(name, shape, dt=BF16):
        return nc.dram_tensor(name, shape, dt, kind="ExternalInput")

    xiT = din("xiT", [IMG_D, R])
    xtT = din("xtT", [TXT_D, R])
    wiT = din("wiT", [IMG_D, H])
    wtT = din("wtT", [TXT_D, H])
    # pre-tiled weights: [P, MT, KT, 128]; slice [:, mt] is one SBUF tile,
    # contiguous per partition (KT*128 elements)
    wvoT = din("wvoT", [P, KT_H, KT_H, P])
    wqT = din("wqT", [P, KT_H, KT_H, P], FP8)
    wqbT = din("wqbT", [P, KT_H, KT_H, P], FP8)
    wkT = din("wkT", [P, KT_H, KT_H, P], FP8)
    wvT = din("wvT", [P, KT_H, KT_H, P], FP8)
    wvbT = din("wvbT", [P, KT_H, KT_H, P], FP8)
    woT = din("woT", [P, KT_H, KT_H, P], FP8)
    w1aT = din("w1aT", [P, KT_F, KT_H, P], FP8)
    w1bT = din("w1bT", [P, KT_F, KT_H, P], FP8)
    w2aT = din("w2aT", [P, KT_H, KT_F, P], FP8)
    w2bT = din("w2bT", [P, KT_H, KT_F, P], FP8)
    gwiT = din("gwiT", [P, KT_H, KT_H, P])
    gwtT = din("gwtT", [P, KT_H, KT_H, P])
    ident_d = din("ident128", [P, P])

    bias_names = ["bi", "bt", "bvo", "sbq", "sbv8", "sbo", "fb2", "gb",
                  "n1g", "n1b", "n2g", "n2b", "n3gf", "n3bf",
                  "fb2n8"]
    NB = len(bias_names) * KT_H + KT_F
    bias_all_d = din("bias_all", [P, NB], F32)
    hmask_d = din("hmask", [P, 2])
    hmaskT_d = din("hmaskT", [34, P])  # rows 0-1 and 32-33 hold hmask.T

    # DRAM spill for imgp/txtp between P3 and P8 (frees SBUF during SA/FFN)
    imgp_d = nc.dram_tensor("imgp_spill", [P, KT_H, R], BF16)
    txtp_d = nc.dram_tensor("txtp_spill", [P, KT_H, R], BF16)

    outT = nc.dram_tensor("outT", [H, R], F32, kind="ExternalOutput")

    with tile.TileContext(nc) as tc:
        def open_pool(**kw):
            cm = tc.tile_pool(**kw)
            return cm, cm.__enter__()

        def scope(name):
            import contextlib

            @contextlib.contextmanager
            def _s():
                sid, _ = nc.enter_named_scope(name, False)
                yield
                nc.leave_named_scope(name, sid, False)
            return _s()

        # -------- constants (whole kernel) --------
        const_cm, const = open_pool(name="const", bufs=1)
        ones128 = const.tile([P, P], BF16)
        nc.vector.memset(ones128[:], 1.0)
        eps_col = const.tile([P, 1], F32)
        nc.vector.memset(eps_col[:], EPS)
        zero_col = const.tile([P, 1], F32)
        nc.vector.memset(zero_col[:], 0.0)
        bias_all = const.tile([P, NB], F32)
        bias_sb = {n: bias_all[:, i * KT_H:(i + 1) * KT_H]
                   for i, n in enumerate(bias_names)}
        fb1_sb = bias_all[:, len(bias_names) * KT_H:]
        hmask_sb = const.tile([P, 2], BF16)
        hmaskT_sb = const.tile([34, P], BF16)
        ident_sb = const.tile([P, P], BF16)
        ones2_f8 = const.tile([P, 2, P], FP8)  # DR ones for fp8 LN stats
        nc.vector.memset(ones2_f8[:], 1.0)

        def load_consts():
            nc.sync.dma_start(bias_all[:], bias_all_d[:, :])
            nc.sync.dma_start(hmask_sb[:], hmask_d[:, :])
            nc.sync.dma_start(hmaskT_sb[:], hmaskT_d[:, :])
            nc.sync.dma_start(ident_sb[:], ident_d[:, :])

        # -------- shared SBUF pools (whole kernel) --------
        wpool_cm, wpool = open_pool(name="wpool", bufs=2)   # "w" 4KB slots x2
        tpool_cm, tpool = open_pool(name="tpool", bufs=6)   # "tmp" 2KB x6
        spool_cm, spool = open_pool(name="spool", bufs=4)   # "small" 2KB x4
        acts_cm, acts = open_pool(name="acts", bufs=1)

        def act_tile(tag, name, dt=BF16, pad16=True):
            shape = [P, KT_H, R]
            pad = None
            if dt == FP8 and pad16:
                pad = [P, KT_H, 2 * R]  # keep the recycled slot at 16KB
            return acts.tile(shape, dt, tag=tag, name=name, padded_shape=pad)

        def sp_tile(name, shape=None, dt=BF16):
            """SP slot is 16KB/partition (bf16 [P, KT_H, R])."""
            shape = shape or [P, KT_H, R]
            pad = None
            if mybir.dt.size(dt) == 1:
                pad = [shape[0], shape[1], shape[2] * 2]
            return acts.tile(shape, dt, tag="SP", name=name, padded_shape=pad)

        pmain = None
        paux = None

        def load_w(w4_d, kt, mt, name):
            """bf16 weight tile [128, kt, 128] from pre-tiled DRAM [:, mt]."""
            if kt > KT_H:
                t = wpool.tile([P, KT_F, P], BF16, tag="w", name=name)
            else:
                t = wpool.tile([P, KT_H, P], BF16, tag="w_h", name=name, bufs=7)
            nc.sync.dma_start(t[:, :kt, :], w4_d[:, mt])
            return t

        def load_w8(w4_d, kt, mt, name):
            """fp8 weight tile [128, kt, 128] (big 'w' slot is 4KB fp8)."""
            if kt > KT_H:
                t = wpool.tile([P, KT_F, P], FP8, tag="w", name=name)
            else:
                t = wpool.tile([P, KT_H, P], FP8, tag="w_h", name=name, bufs=7,
                               padded_shape=[P, KT_H, 2 * P])
            nc.sync.dma_start(t[:, :kt, :], w4_d[:, mt])
            return t

        def mm_dr(ps, wt, act8, kt, ch, start=True, stop=True):
            """fp8 DoubleRow accumulation chain: kt k-tiles as kt//2 pairs."""
            np_ = kt // 2
            for k in range(np_):
                nc.tensor.matmul(
                    ps[:], lhsT=wt[:, 2 * k:2 * k + 2, :],
                    rhs=act8[:, 2 * k:2 * k + 2, _chsl(ch)],
                    start=(start and k == 0), stop=(stop and k == np_ - 1),
                    perf_mode=DR,
                )

        lnp_cm, lnp = open_pool(name="lnp", bufs=2)  # LN stats (mf bf16, ivf f32)

        def _stats_finish(sb_, qb_, s_scale, q_scale):
            mf = lnp.tile([P, CHS], BF16, tag="lnm", name="ln_mf")
            nc.vector.tensor_scalar_mul(mf[:], sb_[:], s_scale)
            msq = tpool.tile([P, CHS], F32, tag="tmp", name="ln_msq")
            nc.vector.tensor_mul(out=msq[:], in0=mf[:], in1=mf[:])
            vf = tpool.tile([P, CHS], F32, tag="tmp", name="ln_vf")
            nc.vector.scalar_tensor_tensor(vf[:], qb_[:], q_scale, msq[:],
                                           op0=ALU.mult, op1=ALU.subtract)
            sd = tpool.tile([P, CHS], F32, tag="tmp", name="ln_sd")
            nc.scalar.activation(sd[:], vf[:], AF.Sqrt, bias=eps_col[:], scale=1.0)
            # ~5x faster than nc.vector.reciprocal (which stalled PE 3.4us)
            ivf_f = tpool.tile([P, CHS], F32, tag="tmp", name="ln_ivf_f")
            nc.vector.reciprocal_approx_fast(out=ivf_f[:], in_=sd[:])
            ivf = lnp.tile([P, CHS], BF16, tag="lni", name="ln_ivf")
            nc.vector.tensor_scalar_mul(ivf[:], ivf_f[:], 1.0)
            return mf, ivf

        def stats_mms(x_bf, x2, ch):
            """LN row mean + rsqrt(var): Sb/Qb ones-matmuls over pre-staged
            x^2 tiles, then the small DVE/ACT finishing chain."""
            sb_ = paux.tile([P, CHS], F32, tag="Sb", name="ln_Sb")
            for k in range(KT_H):
                nc.tensor.matmul(sb_[:], lhsT=ones128[:],
                                 rhs=x_bf[:, k, _chsl(ch)],
                                 start=(k == 0), stop=(k == KT_H - 1))
            qb_ = paux.tile([P, CHS], F32, tag="Qb", name="ln_Qb")
            for k in range(KT_H):
                nc.tensor.matmul(qb_[:], lhsT=ones128[:],
                                 rhs=x2[:, k, _chsl(ch)],
                                 start=(k == 0), stop=(k == KT_H - 1))
            return _stats_finish(sb_, qb_, 1.0 / H, 1.0 / H)

        def stats_mms_dr(xp, ch):
            """fp8 DoubleRow LN stats: xp packs fp8(x) at [:, :, 0:R] and
            fp8(x)^2 at [:, :, R:2R]; half the matmul slots of stats_mms.
            Stat quantization noise is ~0.1% of the LN scale (negligible)."""
            sb_ = paux.tile([P, CHS], F32, tag="Sb", name="ln_Sb8")
            for k in range(KT_H // 2):
                nc.tensor.matmul(sb_[:], lhsT=ones2_f8[:, 0:2, :],
                                 rhs=xp[:, 2 * k:2 * k + 2, ch * CHS:(ch + 1) * CHS],
                                 start=(k == 0), stop=(k == KT_H // 2 - 1),
                                 perf_mode=DR)
            qb_ = paux.tile([P, CHS], F32, tag="Qb", name="ln_Qb8")
            for k in range(KT_H // 2):
                nc.tensor.matmul(qb_[:], lhsT=ones2_f8[:, 0:2, :],
                                 rhs=xp[:, 2 * k:2 * k + 2,
                                        R + ch * CHS:R + (ch + 1) * CHS],
                                 start=(k == 0), stop=(k == KT_H // 2 - 1),
                                 perf_mode=DR)
            return _stats_finish(sb_, qb_, 1.0 / H, 1.0 / H)

        def ln_norm(x_bf, stats, ch, g_name, b_name, out_bf,
                    out_f8=None):
            """Normalize: 3 DVE ops per k-tile; the fp8 copy (= out_bf/8)
            is derived on ACT."""
            g = bias_sb[g_name]
            bb = bias_sb[b_name]
            mf, ivf = stats
            for k in range(KT_H):
                t1 = tpool.tile([P, CHS], BF16, tag="tmp", name="ln_t1")
                nc.vector.tensor_sub(out=t1[:], in0=x_bf[:, k, _chsl(ch)], in1=mf[:])
                t2 = tpool.tile([P, CHS], BF16, tag="tmp", name="ln_t2")
                nc.vector.tensor_mul(out=t2[:], in0=t1[:], in1=ivf[:])
                nc.vector.tensor_scalar(out_bf[:, k, _chsl(ch)], t2[:],
                                        g[:, k:k + 1], bb[:, k:k + 1],
                                        op0=ALU.mult, op1=ALU.add)
                if out_f8 is not None:
                    nc.scalar.activation(
                        out_f8[:, k, _chsl(ch)], out_bf[:, k, _chsl(ch)],
                        AF.Identity, bias=zero_col[:, :], scale=1.0 / 8.0)

        def ln_norm_raw(x_bf, stats, ch, out_bf, out_f8, f8_dve=False):
            """FFN-path normalize: writes the RAW normalized value u into
            out_bf and u/8 into out_f8 (the LN gain/bias are folded into the
            FFN w1 weights host-side). The gain/bias for the bf16 residual
            copy are applied later, in place, off the critical path.
            f8_dve routes the fp8 copy to DVE (keeps ACT free for the FFN
            gelu evictions that run concurrently)."""
            mf, ivf = stats
            for k in range(KT_H):
                t1 = tpool.tile([P, CHS], BF16, tag="tmp", name="ln_t1")
                nc.vector.tensor_sub(out=t1[:], in0=x_bf[:, k, _chsl(ch)], in1=mf[:])
                nc.vector.tensor_mul(out=out_bf[:, k, _chsl(ch)], in0=t1[:],
                                     in1=ivf[:])
                if f8_dve:
                    nc.vector.tensor_scalar_mul(
                        out_f8[:, k, _chsl(ch)], out_bf[:, k, _chsl(ch)],
                        1.0 / 8.0)
                else:
                    nc.scalar.activation(
                        out_f8[:, k, _chsl(ch)], out_bf[:, k, _chsl(ch)],
                        AF.Identity, bias=zero_col[:, :], scale=1.0 / 8.0)

        def ln_gain_inplace(out_bf, ch, g_name, b_name):
            """Deferred: out_bf = out_bf * g + b, in place (DVE)."""
            g = bias_sb[g_name]
            bb = bias_sb[b_name]
            for k in range(KT_H):
                nc.vector.tensor_scalar(out_bf[:, k, _chsl(ch)],
                                        out_bf[:, k, _chsl(ch)],
                                        g[:, k:k + 1], bb[:, k:k + 1],
                                        op0=ALU.mult, op1=ALU.add)

        # ================= P0/P1: input projections (streamed) =============
        imgp = act_tile("S1", "imgp")
        txtp = act_tile("S2", "txtp")

        def input_proj(xT_d, w_d, kt_in, bname, dst, post_dma=None):
            for ch in range(CH):
                pss = [pmain.tile([P, CHS], F32, tag=f"mm{mt}", name=f"ps{mt}")
                       for mt in range(KT_H)]
                for k in range(kt_in):
                    wt = wpool.tile([P, H], BF16, tag="w_h", name="wrow", bufs=7)
                    nc.sync.dma_start(wt[:], w_d[k * P:(k + 1) * P, :])
                    xs = tpool.tile([P, CHS], BF16, tag="tmp", name="xslice")
                    nc.sync.dma_start(xs[:], xT_d[k * P:(k + 1) * P, _chsl(ch)])
                    for mt in range(KT_H):
                        nc.tensor.matmul(pss[mt][:], lhsT=wt[:, mt * P:(mt + 1) * P],
                                         rhs=xs[:], start=(k == 0), stop=(k == kt_in - 1))
                if post_dma is not None:
                    post_dma()
                    post_dma = None
                # alternate eviction engines so the tail of the last chunk
                # drains in half the time (frees PSUM banks for P23 sooner)
                for mt in range(KT_H):
                    if mt % 2 == 0:
                        nc.scalar.activation(dst[:, mt, _chsl(ch)], pss[mt][:],
                                             AF.Identity,
                                             bias=bias_sb[bname][:, mt:mt + 1],
                                             scale=1.0)
                    else:
                        nc.vector.tensor_scalar(dst[:, mt, _chsl(ch)], pss[mt][:],
                                                bias_sb[bname][:, mt:mt + 1], None,
                                                op0=ALU.add)

        with scope("P01"), tc.tile_pool(name="pmm01", bufs=1, space="PSUM") as pmain:
            # HAM warm-up: ~120 tiny matmuls (never read) fill the initial
            # weight-DMA wait so the PE clock is at 2.4 GHz (not the cold
            # 1.2) when the first real matmul issues. Reuses the mm0 bank;
            # the real chain's start=True clears it.
            wps = pmain.tile([P, P], F32, tag="mm0", name="warm")
            for i in range(120):
                nc.tensor.matmul(wps[:], lhsT=ones128[:], rhs=ones128[:],
                                 start=(i == 0), stop=(i == 119))
            input_proj(xiT, wiT, KT_I, "bi", imgp, post_dma=load_consts)
            input_proj(xtT, wtT, KT_T, "bt", txtp)
            nc.sync.dma_start(imgp_d[:, :, :], imgp[:])
            nc.sync.dma_start(txtp_d[:, :, :], txtp[:])
            # prefetch P23's first two weight tiles into the big-weight slots
            wvo_pre = []
            for mt in range(2):
                t = wpool.tile([P, KT_H, P], BF16, tag="w", name=f"wvo_pre{mt}",
                               padded_shape=[P, 2 * KT_H, P])
                nc.sync.dma_start(t[:, :, :], wvoT[:, mt])
                wvo_pre.append(t)

        # ============ P2/P3: merged cross-attention + LN ============
        c0 = act_tile("S3", "c0")
        c1 = act_tile("S4", "c1")
        c0_f8 = act_tile("C8a", "c0_f8", FP8, pad16=False)  # c0/8 for DR rhs
        c1_f8 = act_tile("C8b", "c1_f8", FP8, pad16=False)

        # ONE PSUM scope spans P23+P3g+P4: separate scopes would barrier
        # each phase's first matmul on the previous phase's PSUM retirement
        # (23.6us measured at P23->P3g). P4's score tiles overlay the
        # Sb/Qb stat tags.
        with (
            scope("P23"),
            tc.tile_pool(name="pmm234", bufs=4, space="PSUM") as pmain,
            tc.tile_pool(name="paux234", bufs=2, space="PSUM") as paux,
        ):
            pca_cm, pca = open_pool(name="pca", bufs=1)
            # x0/x1 GEMMs share one wvo load per mt; DVE evicts
            # (bias + residual); x^2 tiles staged into the free S5/S6 slots
            # right after each mt's evictions so the stat MMs never stall.
            x0 = sp_tile("x0")
            x1 = pca.tile([P, KT_H, R], BF16, tag="x1", name="x1")
            x2a = act_tile("S5", "x2a")  # x0^2
            x2b = act_tile("S6", "x2b")  # x1^2
            for mt in range(KT_H):
                wt = wvo_pre[mt] if mt < 2 else load_w(wvoT, KT_H, mt, "wvo")
                for src, res, dst in ((txtp, imgp, x0), (imgp, txtp, x1)):
                    for ch in range(CH):
                        ps = pmain.tile([P, CHS], F32, tag="mm", name="ps_mm")
                        for k in range(KT_H):
                            nc.tensor.matmul(ps[:], lhsT=wt[:, k, :],
                                             rhs=src[:, k, _chsl(ch)],
                                             start=(k == 0), stop=(k == KT_H - 1))
                        nc.vector.scalar_tensor_tensor(
                            dst[:, mt, _chsl(ch)], ps[:],
                            bias_sb["bvo"][:, mt:mt + 1],
                            res[:, mt, _chsl(ch)], op0=ALU.add, op1=ALU.add)
                for x_bf, x2 in ((x0, x2a), (x1, x2b)):
                    for ch in range(CH):
                        nc.vector.tensor_mul(out=x2[:, mt, _chsl(ch)],
                                             in0=x_bf[:, mt, _chsl(ch)],
                                             in1=x_bf[:, mt, _chsl(ch)])
            cd_f8 = sp_tile("cd", dt=FP8)  # (c0-c1)/8, built per tile
            # interleave so the PE stat MMs stay contiguous (paux ring 2
            # frees early via the sm chains) while lnp ring 3 carries the
            # stats across the norm chains
            st_x0c0 = stats_mms(x0, x2a, 0)
            st_x0c1 = stats_mms(x0, x2a, 1)
            ln_norm(x0, st_x0c0, 0, "n1g", "n1b", c0, c0_f8)
            st_x1c0 = stats_mms(x1, x2b, 0)
            ln_norm(x0, st_x0c1, 1, "n1g", "n1b", c0, c0_f8)
            st_x1c1 = stats_mms(x1, x2b, 1)
            for ch, st in ((0, st_x1c0), (1, st_x1c1)):
                ln_norm(x1, st, ch, "n2g", "n2b", c1, c1_f8)
                for k in range(KT_H):
                    nc.vector.tensor_sub(out=cd_f8[:, k, _chsl(ch)],
                                         in0=c0_f8[:, k, _chsl(ch)],
                                         in1=c1_f8[:, k, _chsl(ch)])

            # ============ P3g: gate logits (independent filler) ==========
            # Pure GEMMs with ACT evictions: the PE stays busy here while
            # the LN chains drain on DVE; P4's weights prefetch at the tail.
            g_sb = act_tile("SG", "g_sb")
            sid3g, _ = nc.enter_named_scope("P3g", False)
            qkv_pre = None
            for mt in range(KT_H):
                wgi = load_w(gwiT, KT_H, mt, "wgi")
                wgt = load_w(gwtT, KT_H, mt, "wgt")
                for ch in range(CH):
                    ps = pmain.tile([P, CHS], F32, tag="mm", name="ps_g")
                    for k in range(KT_H):
                        nc.tensor.matmul(ps[:], lhsT=wgi[:, k, :],
                                         rhs=imgp[:, k, _chsl(ch)],
                                         start=(k == 0), stop=False)
                    for k in range(KT_H):
                        nc.tensor.matmul(ps[:], lhsT=wgt[:, k, :],
                                         rhs=txtp[:, k, _chsl(ch)],
                                         start=False, stop=(k == KT_H - 1))
                    # sigmoid fused into the eviction: g_sb holds the gate
                    nc.scalar.activation(g_sb[:, mt, _chsl(ch)], ps[:], AF.Sigmoid,
                                         bias=bias_sb["gb"][:, mt:mt + 1], scale=1.0)
                if mt == KT_H - 2:
                    # prefetch only P4's first-needed (score-path) weights;
                    # 5 DMAs here congested the queue and stalled P3g's tail
                    qkv_pre = [load_w8(w_d, KT_H, 0, nm) for w_d, nm in
                               ((wqT, "wq"), (wkT, "wk"), (wqbT, "wqb"))]
            nc.leave_named_scope("P3g", sid3g, False)

            # ========= P4: self-attention qkv + scores (fp8 DR) ==========
            v0 = act_tile("S5", "v0")   # v/8 (bf16); recycles x2a
            v1 = act_tile("S6", "v1")
            o0 = act_tile("S1", "o0", FP8)  # o/8, after imgp's last read
            o1 = act_tile("S2", "o1", FP8)
            pca_cm.__exit__(None, None, None)  # free x1's 16KB for pqk
            sid4, _ = nc.enter_named_scope("P4", False)
            pqk_cm, pqk = open_pool(name="pqk", bufs=1)

            def qkv8(wt, act8, bname, mt, dst_t, dst_mt=None, scale=1.0,
                     dve=False):
                for ch in range(CH):
                    ps = pmain.tile([P, CHS], F32, tag="mm", name="ps_qkv")
                    mm_dr(ps, wt, act8, KT_H, ch)
                    bias = bias_sb[bname][:, mt:mt + 1] if bname else zero_col[:, :]
                    dst = (dst_t[:, _chsl(ch)] if dst_mt is None
                           else dst_t[:, dst_mt, _chsl(ch)])
                    if dve:
                        # DVE eviction: ACT is the tighter engine in P4
                        nc.vector.tensor_scalar(dst, ps[:], scale, bias,
                                                op0=ALU.mult, op1=ALU.add)
                    else:
                        nc.scalar.activation(dst, ps[:], AF.Identity,
                                             bias=bias, scale=scale)

            hm2 = hmask_sb[:, :]    # [128, 2] local-head one-hot
            AB = float(SA / SB)  # hmaskT_sb[0:2] is the [2,128] broadcast mask

            def gemm_part(mt, wts):
                """GEMMs + the DVE muls that feed the score chain."""
                wq_t, wv_t, wk_t, wqb_t, wvb_t = wts
                q0t = pqk.tile([P, R], BF16, tag="q0t", bufs=2)
                q1t = pqk.tile([P, R], BF16, tag="q1t", bufs=2)
                kdt = pqk.tile([P, R], BF16, tag="kdt")
                qkv8(wq_t, c0_f8, "sbq", mt, q0t)
                qkv8(wk_t, cd_f8, None, mt, kdt)  # k0-k1; bias cancels
                qkv8(wqb_t, c1_f8, "sbq", mt, q1t, scale=AB)
                nc.vector.tensor_mul(out=q0t[:], in0=q0t[:], in1=kdt[:])
                nc.vector.tensor_mul(out=q1t[:], in0=q1t[:], in1=kdt[:])
                qkv8(wv_t, c0_f8, "sbv8", mt, v0, dst_mt=mt, scale=1.0 / 8.0)
                qkv8(wvb_t, c1_f8, "sbv8", mt, v1, dst_mt=mt, scale=1.0 / SB)
                return mt, q0t, q1t

            def tail_part(st):
                """Score matmuls + attention combine for a PREVIOUS mt:
                emitted one iteration late so the cross-engine latency
                chain (evict->mul->dmm->sigmoid->ab) hides under the next
                mt's GEMM stream instead of stalling the PE."""
                mt, m0, m1 = st
                a_ts = {}
                for ch in range(CH):
                    for m_t, nm in ((m0, "A"), (m1, "B")):
                        dmm = paux.tile([2, CHS], F32, tag="Sb", name=f"dmm{nm}")
                        nc.tensor.matmul(dmm[:], lhsT=hm2, rhs=m_t[:, _chsl(ch)],
                                         start=True, stop=True)
                        a_t = spool.tile([2, CHS], BF16, tag="small", name=f"a{nm}")
                        nc.scalar.activation(a_t[:], dmm[:], AF.Sigmoid,
                                             bias=zero_col[0:2, :],
                                             scale=float(1.0 / np.sqrt(HD)))
                        a_ts[(ch, nm)] = a_t
                for ch in range(CH):
                    diff = tpool.tile([P, CHS], BF16, tag="tmp", name="att_diff")
                    nc.vector.tensor_sub(out=diff[:], in0=v0[:, mt, _chsl(ch)],
                                         in1=v1[:, mt, _chsl(ch)])
                    for o_t, nm in ((o0, "A"), (o1, "B")):
                        ab = paux.tile([P, CHS], F32, tag="Qb", name=f"ab{nm}")
                        nc.tensor.matmul(ab[:], lhsT=hmaskT_sb[0:2, :],
                                         rhs=a_ts[(ch, nm)][:],
                                         start=True, stop=True)
                        t_t = tpool.tile([P, CHS], BF16, tag="tmp", name=f"att_t{nm}")
                        nc.vector.tensor_mul(out=t_t[:], in0=diff[:], in1=ab[:])
                        nc.vector.tensor_add(out=o_t[:, mt, _chsl(ch)], in0=t_t[:],
                                             in1=v1[:, mt, _chsl(ch)])

            wq_next = None
            pend = None
            for mt in range(KT_H):
                if mt == 0:
                    wq0, wk0, wqb0 = qkv_pre
                    wts = (wq0,
                           load_w8(wvT, KT_H, 0, "wv"),
                           wk0, wqb0,
                           load_w8(wvbT, KT_H, 0, "wvb"))
                else:
                    wts = (wq_next,
                           load_w8(wvT, KT_H, mt, "wv"),
                           load_w8(wkT, KT_H, mt, "wk"),
                           load_w8(wqbT, KT_H, mt, "wqb"),
                           load_w8(wvbT, KT_H, mt, "wvb"))
                st = gemm_part(mt, wts)
                if mt + 1 < KT_H:
                    wq_next = load_w8(wqT, KT_H, mt + 1, "wq")
                if pend is not None:
                    tail_part(pend)
                pend = st
            tail_part(pend)
            # prefetch P5's first wo tile into the w_h ring
            wo_pre = load_w8(woT, KT_H, 0, "wo")
            nc.leave_named_scope("P4", sid4, False)

            # prefetch the first two FFN w1 tiles into the big-weight slots
            w1_pre = []
            for mt in range(2):
                t = wpool.tile([P, KT_F, P], FP8, tag="w", name=f"w1_pre{mt}")
                nc.sync.dma_start(t[:, :KT_H, :], w1aT[:, mt])
                w1_pre.append(t)

            # ===== P5: SA out-proj (fp8 DR) + residual + LN3 =====
            # Residual folded into PSUM via an identity matmul; ACT evicts.
            # LN3's bf16 output is stored as r + ffn_b2 (n3bf = n3_b + fb2)
            # so the FFN w2 evict needs no extra bias op.
            r0 = act_tile("S1", "r0")    # r0 + fb2 (bf16); reuses o0 slot
            r1 = act_tile("S2", "r1")
            r0_f8 = act_tile("C8a", "r0_f8", FP8, pad16=False)
            r1_f8 = act_tile("C8b", "r1_f8", FP8, pad16=False)
            imgp2 = act_tile("S3", "imgp2")
            txtp2 = act_tile("S4", "txtp2")
            pqk_cm.__exit__(None, None, None)  # free 10KB for psa
            sid5, _ = nc.enter_named_scope("P5", False)
            psa_cm, psa = open_pool(name="psa", bufs=1)
            y0 = sp_tile("y0")
            y1 = psa.tile([P, KT_H, R], BF16, tag="y1", name="y1")
            y2a = act_tile("S5", "y2a")  # y0^2; recycles v0 slot
            y2b = act_tile("S6", "y2b")

            # y0's whole pipeline runs FIRST so its stats/norm chain (the
            # FFN pos0 critical path) starts ~17us earlier; y1's GEMMs then
            # cover the y0 norm DVE work. wo tiles are loaded twice (+1MB).
            def wo_pass(o_t, res, dst, y2, first_pass):
                for mt in range(KT_H):
                    if first_pass and mt == 0:
                        wt = wo_pre
                    else:
                        wt = load_w8(woT, KT_H, mt, "wo")
                    for ch in range(CH):
                        ps = pmain.tile([P, CHS], F32, tag="mm", name="ps_wo")
                        mm_dr(ps, wt, o_t, KT_H, ch, stop=False)
                        nc.tensor.matmul(ps[:], lhsT=ident_sb[:],
                                         rhs=res[:, mt, _chsl(ch)],
                                         start=False, stop=True)
                        nc.scalar.activation(dst[:, mt, _chsl(ch)], ps[:],
                                             AF.Identity,
                                             bias=bias_sb["sbo"][:, mt:mt + 1],
                                             scale=1.0)
                    for ch in range(CH):
                        nc.vector.tensor_mul(out=y2[:, mt, _chsl(ch)],
                                             in0=dst[:, mt, _chsl(ch)],
                                             in1=dst[:, mt, _chsl(ch)])

            wo_pass(o0, c0, y0, y2a, True)
            for ch in range(CH):
                sty = stats_mms(y0, y2a, ch)
                ln_norm_raw(y0, sty, ch, r0, r0_f8)
            wo_pass(o1, c1, y1, y2b, False)
            for ch in range(CH):
                ln_gain_inplace(r0, ch, "n3gf", "n3bf")
            sty1 = [stats_mms(y1, y2b, ch) for ch in range(CH)]
            for ch in range(CH):
                ln_norm_raw(y1, sty1[ch], ch, r1, r1_f8, f8_dve=True)
                ln_gain_inplace(r1, ch, "n3gf", "n3bf")

            # ===== P6/P7: FFN both positions (fp8 DR); pooled accumulation
            # SAME pool scope as P5: a separate scope would barrier the FFN
            # start on the y1-stats PSUM retirement (9.5us measured).
            # mt-outer / ch-inner: each weight tile loaded once per position.
            pooled = sp_tile("pooled")  # bf16; pos1 fuses the final combine
            sid67, _ = nc.enter_named_scope("P67", False)
            gate_done = False
            for pos, (r_f8, r_p, w1d, w2d, first) in enumerate([
                    (r0_f8, r0, w1aT, w2aT, True),
                    (r1_f8, r1, w1bT, w2bT, False)]):
                if not first and not gate_done:
                    # g_sb <- gate*(imgp-txtp) + txtp in place; runs on DVE
                    # during pos0's PE work so the pos1 chain is 3 ops + DMA
                    gate_done = True
                    for gmt in range(KT_H):
                        for gch in range(CH):
                            gsl = _chsl(gch)
                            gd = tpool.tile([P, CHS], BF16, tag="tmp", name="gd")
                            nc.vector.tensor_sub(out=gd[:],
                                                 in0=imgp2[:, gmt, gsl],
                                                 in1=txtp2[:, gmt, gsl])
                            gt = tpool.tile([P, CHS], BF16, tag="tmp", name="gt")
                            nc.vector.tensor_mul(out=gt[:],
                                                 in0=g_sb[:, gmt, gsl], in1=gd[:])
                            nc.vector.tensor_add(out=g_sb[:, gmt, gsl],
                                                 in0=gt[:], in1=txtp2[:, gmt, gsl])
                            # fold the gate term into pooled here (DVE has
                            # slack under pos1's w1 GEMMs): shortens the
                            # per-(mt,ch) output chain to 2 ops
                            nc.vector.tensor_add(out=pooled[:, gmt, gsl],
                                                 in0=pooled[:, gmt, gsl],
                                                 in1=g_sb[:, gmt, gsl])
                gelu_scale = 1.0 if first else float(SA / SB)
                ev_scale = (1.0 / SA) if first else (1.0 / SB)
                # hidden [128, 16, 1024] fp8 x2 in the freed y2a/y2b slots
                h_a = acts.tile([P, KT_F // 2, R], FP8, tag="S5",
                                name=f"h_a{pos}")
                h_b = acts.tile([P, KT_F // 2, R], FP8, tag="S6",
                                name=f"h_b{pos}")
                if first:
                    # start with ch0-only chains: r0_f8[ch1] finishes on DVE
                    # a few us after ch0, so ch1 work is deferred past it
                    # (mt0/1 sit in the big w slots, so all 8 fit the ring)
                    NCH0 = 8
                    w1_order = ([(mt, 0) for mt in range(NCH0)]
                                + [(mt, 1) for mt in range(NCH0)]
                                + [(mt, ch) for mt in range(NCH0, KT_F)
                                   for ch in range(CH)])
                else:
                    w1_order = [(mt, ch) for mt in range(KT_F)
                                for ch in range(CH)]
                w1_tiles = {}
                for mt, ch in w1_order:
                    if mt not in w1_tiles:
                        if first and mt < 2:
                            w1_tiles[mt] = w1_pre[mt]
                        else:
                            w1_tiles[mt] = load_w8(w1d, KT_H, mt, "w1")
                    if first and ch == 0:
                        # reload the P8 gate operands (4MB) here, past the
                        # FFN head, so they don't contend with the w1 stream
                        if mt == 10:
                            nc.sync.dma_start(imgp2[:], imgp_d[:, :, :])
                        elif mt == 20:
                            nc.sync.dma_start(txtp2[:], txtp_d[:, :, :])
                    wt = w1_tiles[mt]
                    hdst = h_a if mt < KT_F // 2 else h_b
                    ps = pmain.tile([P, CHS], F32, tag="mm", name="ps_f1")
                    mm_dr(ps, wt, r_f8, KT_H, ch)
                    nc.scalar.activation(
                        hdst[:, mt % (KT_F // 2), _chsl(ch)], ps[:],
                        AF.Gelu, bias=fb1_sb[:, mt:mt + 1], scale=gelu_scale)
                for mt in range(KT_H):
                    wt = load_w8(w2d, KT_F, mt, "w2")
                    for ch in range(CH):
                        ps = pmain.tile([P, CHS], F32, tag="mm", name="ps_f2")
                        for k in range(KT_F // 4):
                            nc.tensor.matmul(
                                ps[:], lhsT=wt[:, 2 * k:2 * k + 2, :],
                                rhs=h_a[:, 2 * k:2 * k + 2, _chsl(ch)],
                                start=(k == 0), stop=False, perf_mode=DR)
                        for k in range(KT_F // 4):
                            nc.tensor.matmul(
                                ps[:],
                                lhsT=wt[:, KT_F // 2 + 2 * k:KT_F // 2 + 2 * k + 2, :],
                                rhs=h_b[:, 2 * k:2 * k + 2, _chsl(ch)],
                                start=False, stop=(k == KT_F // 4 - 1),
                                perf_mode=DR)
                        if first:
                            # r0 holds 0.5*(r+fb2): pooled = 0.5*p0
                            nc.vector.scalar_tensor_tensor(
                                pooled[:, mt, _chsl(ch)], ps[:], 0.5 * ev_scale,
                                r_p[:, mt, _chsl(ch)], op0=ALU.mult, op1=ALU.add)
                        else:
                            # final combine (gate pre-added into pooled):
                            # out = [0.5*p1] + [0.5*p0 + gate-term]
                            tmp = tpool.tile([P, CHS], F32, tag="tmp", name="ffn_tmp")
                            nc.vector.scalar_tensor_tensor(
                                tmp[:], ps[:], 0.5 * ev_scale,
                                r_p[:, mt, _chsl(ch)], op0=ALU.mult, op1=ALU.add)
                            fin = tpool.tile([P, CHS], F32, tag="tmp", name="gfin")
                            nc.vector.tensor_add(out=fin[:], in0=tmp[:],
                                                 in1=pooled[:, mt, _chsl(ch)])
                            nc.sync.dma_start(outT[mt * P:(mt + 1) * P, _chsl(ch)],
                                              fin[:])
            nc.leave_named_scope("P67", sid67, False)
            nc.leave_named_scope("P5", sid5, False)
            psa_cm.__exit__(None, None, None)

        lnp_cm.__exit__(None, None, None)
        acts_cm.__exit__(None, None, None)
        spool_cm.__exit__(None, None, None)
        tpool_cm.__exit__(None, None, None)
        wpool_cm.__exit__(None, None, None)
        const_cm.__exit__(None, None, None)

    nc.compile()
    return nc


def host_prep(inputs):
    """Host-side preprocessing: merge CA weights, transpose, cast, shard."""
    f = {k: np.asarray(v, dtype=np.float32) for k, v in inputs.items()}

    def bf(x):
        return np.ascontiguousarray(x).astype(np_bf16)

    def tile4(wT, np_dt):
        """[K, M] (already transposed) -> [P, MT, KT, P] pre-tiled layout."""
        K, M = wT.shape
        kt, mtn = K // P, M // P
        w4 = wT.reshape(kt, P, mtn, P).transpose(1, 2, 0, 3)
        return np.ascontiguousarray(w4).astype(np_dt)

    def q8t(w, s):
        """quantize w.T at scale s, pre-tiled."""
        return tile4(np.asarray(w, np.float32).T * s, np_fp8)

    def bft(w):
        return tile4(np.asarray(w, np.float32).T, np_bf16)

    def bias128(x, kt):
        return np.ascontiguousarray(np.asarray(x, np.float32).reshape(kt, P).T)

    ca_wv = np.split(f["ca_wqkv"], 3, axis=0)[2]
    ca_bv = f["ca_bqkv"][2 * H:]
    w_vo = f["ca_wo"] @ ca_wv
    b_vo = f["ca_wo"] @ ca_bv + f["ca_bo"]

    sa_wq, sa_wk, sa_wv = np.split(f["sa_wqkv"], 3, axis=0)
    sa_bq, sa_bk, sa_bv = np.split(f["sa_bqkv"], 3)

    gwi = f["gate_w"][:, :H]
    gwt = f["gate_w"][:, H:]

    # LN3 gain/bias fold for the FFN first layer (see kernel docstring)
    w1f = f["ffn_w1"] * f["n3_g"][None, :]
    fb1f = f["ffn_b1"] + f["ffn_w1"] @ f["n3_b"]

    lh = np.arange(P) // HD  # local head index within a 128-feature tile
    hmask = np.ascontiguousarray((lh[:, None] == np.arange(2)[None, :]).astype(np_bf16))
    hmaskT = np.zeros((34, P), np_bf16)  # copies at rows 0-1 and 32-33
    hmaskT[0:2] = hmask.T
    hmaskT[32:34] = hmask.T
    hmaskT = np.ascontiguousarray(hmaskT)

    shared = {
        "wiT": bf(f["Wi"].T), "wtT": bf(f["Wt"].T),
        "wvoT": bft(w_vo),
        "wqT": q8t(sa_wq, SA), "wqbT": q8t(sa_wq, SB),
        "wkT": q8t(sa_wk, SA),
        "wvT": q8t(sa_wv, SA), "wvbT": q8t(sa_wv, SB),
        "woT": q8t(f["sa_wo"], SA),
        # LN3 gain/bias folded into w1 (input-column scale) and its bias:
        # gelu(u@(w1*g).T + b1 + w1@b) == gelu((u*g+b)@w1.T + b1)
        "w1aT": q8t(w1f, SA), "w1bT": q8t(w1f, SB),
        "w2aT": q8t(f["ffn_w2"], SA), "w2bT": q8t(f["ffn_w2"], SB),
        "gwiT": bft(gwi), "gwtT": bft(gwt),
        "ident128": np.ascontiguousarray(np.eye(P, dtype=np_bf16)),
        "bias_all": np.concatenate([
            bias128(f["bi"], KT_H), bias128(f["bt"], KT_H), bias128(b_vo, KT_H),
            bias128(sa_bq, KT_H), bias128(sa_bv / 8.0, KT_H),
            bias128(f["sa_bo"], KT_H), bias128(f["ffn_b2"], KT_H),
            bias128(f["gate_b"], KT_H),
            bias128(f["n1_g"], KT_H), bias128(f["n1_b"], KT_H),
            bias128(f["n2_g"], KT_H), bias128(f["n2_b"], KT_H),
            bias128(0.5 * f["n3_g"], KT_H),
            bias128(0.5 * (f["n3_b"] + f["ffn_b2"]), KT_H),
            bias128(-f["ffn_b2"] / 8.0, KT_H),
            bias128(fb1f, KT_F),
        ], axis=1),
        "hmask": np.ascontiguousarray(hmask), "hmaskT": hmaskT,
    }

    xiT = f["image_features"].T.astype(np_bf16)  # [IMG_D, B]
    xtT = f["text_features"].T.astype(np_bf16)
    in_maps = []
    for c in range(N_CORES):
        m = dict(shared)
        m["xiT"] = np.ascontiguousarray(xiT[:, c * R:(c + 1) * R])
        m["xtT"] = np.ascontiguousarray(xtT[:, c * R:(c + 1) * R])
        in_maps.append(m)
    return in_maps


_NC_CACHE = None


def kernel(**inputs) -> np.ndarray:
    global _NC_CACHE
    if _NC_CACHE is None:
        _NC_CACHE = build()
    nc = _NC_CACHE
    in_maps = host_prep(inputs)
    res = run_bass_kernel_spmd(nc, in_maps, core_ids=list(range(N_CORES)))
    out = np.empty((B, H), np.float32)
    for c in range(N_CORES):
        out[c * R:(c + 1) * R, :] = res.results[c]["outT"].T
    return out


if __name__ == "__main__":
    nc = build()
    print("built OK")


# revision 63
# speedup vs baseline: 1.0025x; 1.0025x over previous
"""Trainium2 Bass kernel for nn_AttentionFusion (dense transformer block).

Data-parallel over batch: B=8192 rows sharded as 1024 rows per NeuronCore
across 8 cores; weights replicated. On-chip layout is feature-major:
activations are stored as [128 partitions(features), k_tiles, 1024 rows],
so every matmul is out.T[m,n] = sum_k W.T[k,m] * act.T[k,n] with natural
(host-pre-transposed) weight loads and the contraction on the partition dim.

Algebraic simplifications (validated against the reference to 2e-6):
  - Cross-attention has seq len 1 -> softmax == 1 -> out = v @ wo.T + bo;
    additionally (v @ wv.T) @ wo.T = v @ (wo@wv).T is merged on the host.
  - Self-attention has seq len 2 -> softmax([a,b]) = [sig(a-b), 1-sig(a-b)].
  - LayerNorm / attention-score reductions over features (= partitions) are
    done with small matmuls against ones/head-mask matrices.

fp8 (e4m3) DoubleRow matmuls (2x contraction per instruction):
  - SA q/k/v, SA out-proj, FFN w1/w2 run as fp8 DoubleRow.
  - Weights host-quantized at 8x scale (12x for the second FFN position to
    decorrelate quantization noise between the two pooled positions);
    activations stored as value/8 in fp8 so PSUM results are at true scale.
  - Cross-attention / gate / input projections stay bf16 (noise budget).

Scheduling (655us -> 610us measured): every matmul streams 512 columns
(~216 ns) regardless of dtype (DoubleRow's 2x is the doubled contraction
per instruction), so the wins are (a) keeping the in-order PE queue free
of instructions that wait on DVE/ACT, and (b) never closing a PSUM pool
mid-kernel:
  - weights host-pre-tiled [P, MT, KT, 128]: each weight-tile DMA is one
    contiguous per-partition run (no 128B-chunk gathers).
  - ONE PSUM pool scope spans P23..P67 (a scope boundary barriers the next
    phase's first matmul on the previous phase's PSUM retirement).
  - P23: x0/x1 GEMMs share one wvo load per mt; DVE evicts (bias+residual);
    x^2 staged into the free S5/S6 slots; stat MMs emitted after the GEMMs
    (interleaved so lnp ring 2 suffices); the independent gate GEMMs (P3g,
    sigmoid fused into the ACT eviction) cover the LN chains on DVE.
  - LN fp8 copies derived from the bf16 value on ACT (out/8); for LN3 the
    gain/bias are folded into w1/b1 host-side so the FFN consumes the raw
    normalized value (gain applied later, in place, off the critical path).
  - P4 software-pipelined by one mt: the score chain (evict->mul->dmm->
    sigmoid->ab) of mt runs under mt+1's GEMM stream.
  - P5: residual-add folded into PSUM via an identity matmul, ACT evicts;
    y0's whole pipeline (GEMMs->stats->norm) runs before y1's GEMMs so
    r0_f8 is ready when the FFN starts (which begins ch0-only);
    LN3-y1 and the imgp/txtp reload hide under FFN pos0.
  - output chain: r bf16 copies stored pre-halved and the gate term
    pre-added into pooled, so each pos1 eviction is 2 DVE ops + DMA.
"""

import numpy as np
import ml_dtypes

import concourse.bacc as bacc
import concourse.mybir as mybir
import concourse.tile as tile
from concourse.bass_utils import run_bass_kernel_spmd

AF = mybir.ActivationFunctionType
ALU = mybir.AluOpType
BF16 = mybir.dt.bfloat16
F32 = mybir.dt.float32
FP8 = mybir.dt.float8e4
DR = mybir.MatmulPerfMode.DoubleRow

N_CORES = 8
B, IMG_D, TXT_D, H, NH = 8192, 1280, 2048, 1024, 16
HD = H // NH  # 64 head dim
R = B // N_CORES  # 1024 rows per core
P = 128
CH = 2  # row chunks per core
CHS = R // CH  # 512 rows per chunk
KT_I, KT_T, KT_H, KT_F = IMG_D // P, TXT_D // P, H // P, 4 * H // P
EPS = 1e-5
SA = 8.0    # fp8 weight scale, position 0 / shared
SB = 12.0   # fp8 weight scale, position 1 (FFN dual-quantization)

np_bf16 = ml_dtypes.bfloat16
np_fp8 = ml_dtypes.float8_e4m3

def _chsl(ch):
    return slice(ch * CHS, (ch + 1) * CHS)


def build():
    nc = bacc.Bacc(None, target_bir_lowering=False)

    def din(name, shape, dt=BF16):
        return nc.dram_tensor(name, shape, dt, kind="ExternalInput")

    xiT = din("xiT", [IMG_D, R])
    xtT = din("xtT", [TXT_D, R])
    wiT = din("wiT", [IMG_D, H])
    wtT = din("wtT", [TXT_D, H])
    # pre-tiled weights: [P, MT, KT, 128]; slice [:, mt] is one SBUF tile,
    # contiguous per partition (KT*128 elements)
    wvoT = din("wvoT", [P, KT_H, KT_H, P])
    wqT = din("wqT", [P, KT_H, KT_H, P], FP8)
    wqbT = din("wqbT", [P, KT_H, KT_H, P], FP8)
    wkT = din("wkT", [P, KT_H, KT_H, P], FP8)
    wvT = din("wvT", [P, KT_H, KT_H, P], FP8)
    wvbT = din("wvbT", [P, KT_H, KT_H, P], FP8)
    woT = din("woT", [P, KT_H, KT_H, P], FP8)
    w1aT = din("w1aT", [P, KT_F, KT_H, P], FP8)
    w1bT = din("w1bT", [P, KT_F, KT_H, P], FP8)
    w2aT = din("w2aT", [P, KT_H, KT_F, P], FP8)
    w2bT = din("w2bT", [P, KT_H, KT_F, P], FP8)
    gwiT = din("gwiT", [P, KT_H, KT_H, P])
    gwtT = din("gwtT", [P, KT_H, KT_H, P])
    ident_d = din("ident128", [P, P])

    bias_names = ["bi", "bt", "bvo", "sbq", "sbv8", "sbo", "fb2", "gb",
                  "n1g", "n1b", "n2g", "n2b", "n3gf", "n3bf",
                  "fb2n8"]
    NB = len(bias_names) * KT_H + KT_F
    bias_all_d = din("bias_all", [P, NB], F32)
    hmask_d = din("hmask", [P, 2])
    hmaskT_d = din("hmaskT", [34, P])  # rows 0-1 and 32-33 hold hmask.T

    # DRAM spill for imgp/txtp between P3 and P8 (frees SBUF during SA/FFN)
    imgp_d = nc.dram_tensor("imgp_spill", [P, KT_H, R], BF16)
    txtp_d = nc.dram_tensor("txtp_spill", [P, KT_H, R], BF16)

    outT = nc.dram_tensor("outT", [H, R], F32, kind="ExternalOutput")

    with tile.TileContext(nc) as tc:
        def open_pool(**kw):
            cm = tc.tile_pool(**kw)
            return cm, cm.__enter__()

        def scope(name):
            import contextlib

            @contextlib.contextmanager
            def _s():
                sid, _ = nc.enter_named_scope(name, False)
                yield
                nc.leave_named_scope(name, sid, False)
            return _s()

        # -------- constants (whole kernel) --------
        const_cm, const = open_pool(name="const", bufs=1)
        ones128 = const.tile([P, P], BF16)
        nc.vector.memset(ones128[:], 1.0)
        eps_col = const.tile([P, 1], F32)
        nc.vector.memset(eps_col[:], EPS)
        zero_col = const.tile([P, 1], F32)
        nc.vector.memset(zero_col[:], 0.0)
        bias_all = const.tile([P, NB], F32)
        bias_sb = {n: bias_all[:, i * KT_H:(i + 1) * KT_H]
                   for i, n in enumerate(bias_names)}
        fb1_sb = bias_all[:, len(bias_names) * KT_H:]
        hmask_sb = const.tile([P, 2], BF16)
        hmaskT_sb = const.tile([34, P], BF16)
        ident_sb = const.tile([P, P], BF16)
        ones2_f8 = const.tile([P, 2, P], FP8)  # DR ones for fp8 LN stats
        nc.vector.memset(ones2_f8[:], 1.0)

        def load_consts():
            nc.sync.dma_start(bias_all[:], bias_all_d[:, :])
            nc.sync.dma_start(hmask_sb[:], hmask_d[:, :])
            nc.sync.dma_start(hmaskT_sb[:], hmaskT_d[:, :])
            nc.sync.dma_start(ident_sb[:], ident_d[:, :])

        # -------- shared SBUF pools (whole kernel) --------
        wpool_cm, wpool = open_pool(name="wpool", bufs=2)   # "w" 4KB slots x2
        tpool_cm, tpool = open_pool(name="tpool", bufs=6)   # "tmp" 2KB x6
        spool_cm, spool = open_pool(name="spool", bufs=4)   # "small" 2KB x4
        acts_cm, acts = open_pool(name="acts", bufs=1)

        def act_tile(tag, name, dt=BF16, pad16=True):
            shape = [P, KT_H, R]
            pad = None
            if dt == FP8 and pad16:
                pad = [P, KT_H, 2 * R]  # keep the recycled slot at 16KB
            return acts.tile(shape, dt, tag=tag, name=name, padded_shape=pad)

        def sp_tile(name, shape=None, dt=BF16):
            """SP slot is 16KB/partition (bf16 [P, KT_H, R])."""
            shape = shape or [P, KT_H, R]
            pad = None
            if mybir.dt.size(dt) == 1:
                pad = [shape[0], shape[1], shape[2] * 2]
            return acts.tile(shape, dt, tag="SP", name=name, padded_shape=pad)

        pmain = None
        paux = None

        def load_w(w4_d, kt, mt, name):
            """bf16 weight tile [128, kt, 128] from pre-tiled DRAM [:, mt]."""
            if kt > KT_H:
                t = wpool.tile([P, KT_F, P], BF16, tag="w", name=name)
            else:
                t = wpool.tile([P, KT_H, P], BF16, tag="w_h", name=name, bufs=7)
            nc.sync.dma_start(t[:, :kt, :], w4_d[:, mt])
            return t

        def load_w8(w4_d, kt, mt, name):
            """fp8 weight tile [128, kt, 128] (big 'w' slot is 4KB fp8)."""
            if kt > KT_H:
                t = wpool.tile([P, KT_F, P], FP8, tag="w", name=name)
            else:
                t = wpool.tile([P, KT_H, P], FP8, tag="w_h", name=name, bufs=7,
                               padded_shape=[P, KT_H, 2 * P])
            nc.sync.dma_start(t[:, :kt, :], w4_d[:, mt])
            return t

        def mm_dr(ps, wt, act8, kt, ch, start=True, stop=True):
            """fp8 DoubleRow accumulation chain: kt k-tiles as kt//2 pairs."""
            np_ = kt // 2
            for k in range(np_):
                nc.tensor.matmul(
                    ps[:], lhsT=wt[:, 2 * k:2 * k + 2, :],
                    rhs=act8[:, 2 * k:2 * k + 2, _chsl(ch)],
                    start=(start and k == 0), stop=(stop and k == np_ - 1),
                    perf_mode=DR,
                )

        lnp_cm, lnp = open_pool(name="lnp", bufs=2)  # LN stats (mf bf16, ivf f32)

        def _stats_finish(sb_, qb_, s_scale, q_scale):
            mf = lnp.tile([P, CHS], BF16, tag="lnm", name="ln_mf")
            nc.vector.tensor_scalar_mul(mf[:], sb_[:], s_scale)
            msq = tpool.tile([P, CHS], F32, tag="tmp", name="ln_msq")
            nc.vector.tensor_mul(out=msq[:], in0=mf[:], in1=mf[:])
            vf = tpool.tile([P, CHS], F32, tag="tmp", name="ln_vf")
            nc.vector.scalar_tensor_tensor(vf[:], qb_[:], q_scale, msq[:],
                                           op0=ALU.mult, op1=ALU.subtract)
            sd = tpool.tile([P, CHS], F32, tag="tmp", name="ln_sd")
            nc.scalar.activation(sd[:], vf[:], AF.Sqrt, bias=eps_col[:], scale=1.0)
            # ~5x faster than nc.vector.reciprocal (which stalled PE 3.4us)
            ivf_f = tpool.tile([P, CHS], F32, tag="tmp", name="ln_ivf_f")
            nc.vector.reciprocal_approx_fast(out=ivf_f[:], in_=sd[:])
            ivf = lnp.tile([P, CHS], BF16, tag="lni", name="ln_ivf")
            nc.vector.tensor_scalar_mul(ivf[:], ivf_f[:], 1.0)
            return mf, ivf

        def stats_mms(x_bf, x2, ch):
            """LN row mean + rsqrt(var): Sb/Qb ones-matmuls over pre-staged
            x^2 tiles, then the small DVE/ACT finishing chain."""
            sb_ = paux.tile([P, CHS], F32, tag="Sb", name="ln_Sb")
            for k in range(KT_H):
                nc.tensor.matmul(sb_[:], lhsT=ones128[:],
                                 rhs=x_bf[:, k, _chsl(ch)],
                                 start=(k == 0), stop=(k == KT_H - 1))
            qb_ = paux.tile([P, CHS], F32, tag="Qb", name="ln_Qb")
            for k in range(KT_H):
                nc.tensor.matmul(qb_[:], lhsT=ones128[:],
                                 rhs=x2[:, k, _chsl(ch)],
                                 start=(k == 0), stop=(k == KT_H - 1))
            return _stats_finish(sb_, qb_, 1.0 / H, 1.0 / H)

        def stats_mms_dr(xp, ch):
            """fp8 DoubleRow LN stats: xp packs fp8(x) at [:, :, 0:R] and
            fp8(x)^2 at [:, :, R:2R]; half the matmul slots of stats_mms.
            Stat quantization noise is ~0.1% of the LN scale (negligible)."""
            sb_ = paux.tile([P, CHS], F32, tag="Sb", name="ln_Sb8")
            for k in range(KT_H // 2):
                nc.tensor.matmul(sb_[:], lhsT=ones2_f8[:, 0:2, :],
                                 rhs=xp[:, 2 * k:2 * k + 2, ch * CHS:(ch + 1) * CHS],
                                 start=(k == 0), stop=(k == KT_H // 2 - 1),
                                 perf_mode=DR)
            qb_ = paux.tile([P, CHS], F32, tag="Qb", name="ln_Qb8")
            for k in range(KT_H // 2):
                nc.tensor.matmul(qb_[:], lhsT=ones2_f8[:, 0:2, :],
                                 rhs=xp[:, 2 * k:2 * k + 2,
                                        R + ch * CHS:R + (ch + 1) * CHS],
                                 start=(k == 0), stop=(k == KT_H // 2 - 1),
                                 perf_mode=DR)
            return _stats_finish(sb_, qb_, 1.0 / H, 1.0 / H)

        def ln_norm(x_bf, stats, ch, g_name, b_name, out_bf,
                    out_f8=None):
            """Normalize: 3 DVE ops per k-tile; the fp8 copy (= out_bf/8)
            is derived on ACT."""
            g = bias_sb[g_name]
            bb = bias_sb[b_name]
            mf, ivf = stats
            for k in range(KT_H):
                t1 = tpool.tile([P, CHS], BF16, tag="tmp", name="ln_t1")
                nc.vector.tensor_sub(out=t1[:], in0=x_bf[:, k, _chsl(ch)], in1=mf[:])
                t2 = tpool.tile([P, CHS], BF16, tag="tmp", name="ln_t2")
                nc.vector.tensor_mul(out=t2[:], in0=t1[:], in1=ivf[:])
                nc.vector.tensor_scalar(out_bf[:, k, _chsl(ch)], t2[:],
                                        g[:, k:k + 1], bb[:, k:k + 1],
                                        op0=ALU.mult, op1=ALU.add)
                if out_f8 is not None:
                    nc.scalar.activation(
                        out_f8[:, k, _chsl(ch)], out_bf[:, k, _chsl(ch)],
                        AF.Identity, bias=zero_col[:, :], scale=1.0 / 8.0)

        def ln_norm_raw(x_bf, stats, ch, out_bf, out_f8, f8_dve=False):
            """FFN-path normalize: writes the RAW normalized value u into
            out_bf and u/8 into out_f8 (the LN gain/bias are folded into the
            FFN w1 weights host-side). The gain/bias for the bf16 residual
            copy are applied later, in place, off the critical path.
            f8_dve routes the fp8 copy to DVE (keeps ACT free for the FFN
            gelu evictions that run concurrently)."""
            mf, ivf = stats
            for k in range(KT_H):
                t1 = tpool.tile([P, CHS], BF16, tag="tmp", name="ln_t1")
                nc.vector.tensor_sub(out=t1[:], in0=x_bf[:, k, _chsl(ch)], in1=mf[:])
                nc.vector.tensor_mul(out=out_bf[:, k, _chsl(ch)], in0=t1[:],
                                     in1=ivf[:])
                if f8_dve:
                    nc.vector.tensor_scalar_mul(
                        out_f8[:, k, _chsl(ch)], out_bf[:, k, _chsl(ch)],
                        1.0 / 8.0)
                else:
                    nc.scalar.activation(
                        out_f8[:, k, _chsl(ch)], out_bf[:, k, _chsl(ch)],
                        AF.Identity, bias=zero_col[:, :], scale=1.0 / 8.0)

        def ln_gain_inplace(out_bf, ch, g_name, b_name):
            """Deferred: out_bf = out_bf * g + b, in place (DVE)."""
            g = bias_sb[g_name]
            bb = bias_sb[b_name]
            for k in range(KT_H):
                nc.vector.tensor_scalar(out_bf[:, k, _chsl(ch)],
                                        out_bf[:, k, _chsl(ch)],
                                        g[:, k:k + 1], bb[:, k:k + 1],
                                        op0=ALU.mult, op1=ALU.add)

        # ================= P0/P1: input projections (streamed) =============
        imgp = act_tile("S1", "imgp")
        txtp = act_tile("S2", "txtp")

        def input_proj(xT_d, w_d, kt_in, bname, dst, post_dma=None):
            for ch in range(CH):
                pss = [pmain.tile([P, CHS], F32, tag=f"mm{mt}", name=f"ps{mt}")
                       for mt in range(KT_H)]
                for k in range(kt_in):
                    wt = wpool.tile([P, H], BF16, tag="w_h", name="wrow", bufs=7)
                    nc.sync.dma_start(wt[:], w_d[k * P:(k + 1) * P, :])
                    xs = tpool.tile([P, CHS], BF16, tag="tmp", name="xslice")
                    nc.sync.dma_start(xs[:], xT_d[k * P:(k + 1) * P, _chsl(ch)])
                    for mt in range(KT_H):
                        nc.tensor.matmul(pss[mt][:], lhsT=wt[:, mt * P:(mt + 1) * P],
                                         rhs=xs[:], start=(k == 0), stop=(k == kt_in - 1))
                if post_dma is not None:
                    post_dma()
                    post_dma = None
                # alternate eviction engines so the tail of the last chunk
                # drains in half the time (frees PSUM banks for P23 sooner)
                for mt in range(KT_H):
                    if mt % 2 == 0:
                        nc.scalar.activation(dst[:, mt, _chsl(ch)], pss[mt][:],
                                             AF.Identity,
                                             bias=bias_sb[bname][:, mt:mt + 1],
                                             scale=1.0)
                    else:
                        nc.vector.tensor_scalar(dst[:, mt, _chsl(ch)], pss[mt][:],
                                                bias_sb[bname][:, mt:mt + 1], None,
                                                op0=ALU.add)

        with scope("P01"), tc.tile_pool(name="pmm01", bufs=1, space="PSUM") as pmain:
            # HAM warm-up: ~120 tiny matmuls (never read) fill the initial
            # weight-DMA wait so the PE clock is at 2.4 GHz (not the cold
            # 1.2) when the first real matmul issues. Reuses the mm0 bank;
            # the real chain's start=True clears it.
            wps = pmain.tile([P, P], F32, tag="mm0", name="warm")
            for i in range(28):
                nc.tensor.matmul(wps[:], lhsT=ones128[:], rhs=ones128[:],
                                 start=(i == 0), stop=(i == 27))
            input_proj(xiT, wiT, KT_I, "bi", imgp, post_dma=load_consts)
            input_proj(xtT, wtT, KT_T, "bt", txtp)
            nc.sync.dma_start(imgp_d[:, :, :], imgp[:])
            nc.sync.dma_start(txtp_d[:, :, :], txtp[:])
            # prefetch P23's first two weight tiles into the big-weight slots
            wvo_pre = []
            for mt in range(2):
                t = wpool.tile([P, KT_H, P], BF16, tag="w", name=f"wvo_pre{mt}",
                               padded_shape=[P, 2 * KT_H, P])
                nc.sync.dma_start(t[:, :, :], wvoT[:, mt])
                wvo_pre.append(t)

        # ============ P2/P3: merged cross-attention + LN ============
        c0 = act_tile("S3", "c0")
        c1 = act_tile("S4", "c1")
        c0_f8 = act_tile("C8a", "c0_f8", FP8, pad16=False)  # c0/8 for DR rhs
        c1_f8 = act_tile("C8b", "c1_f8", FP8, pad16=False)

        # ONE PSUM scope spans P23+P3g+P4: separate scopes would barrier
        # each phase's first matmul on the previous phase's PSUM retirement
        # (23.6us measured at P23->P3g). P4's score tiles overlay the
        # Sb/Qb stat tags.
        with (
            scope("P23"),
            tc.tile_pool(name="pmm234", bufs=4, space="PSUM") as pmain,
            tc.tile_pool(name="paux234", bufs=2, space="PSUM") as paux,
        ):
            pca_cm, pca = open_pool(name="pca", bufs=1)
            # x0/x1 GEMMs share one wvo load per mt; DVE evicts
            # (bias + residual); x^2 tiles staged into the free S5/S6 slots
            # right after each mt's evictions so the stat MMs never stall.
            x0 = sp_tile("x0")
            x1 = pca.tile([P, KT_H, R], BF16, tag="x1", name="x1")
            x2a = act_tile("S5", "x2a")  # x0^2
            x2b = act_tile("S6", "x2b")  # x1^2
            for mt in range(KT_H):
                wt = wvo_pre[mt] if mt < 2 else load_w(wvoT, KT_H, mt, "wvo")
                for src, res, dst in ((txtp, imgp, x0), (imgp, txtp, x1)):
                    for ch in range(CH):
                        ps = pmain.tile([P, CHS], F32, tag="mm", name="ps_mm")
                        for k in range(KT_H):
                            nc.tensor.matmul(ps[:], lhsT=wt[:, k, :],
                                             rhs=src[:, k, _chsl(ch)],
                                             start=(k == 0), stop=(k == KT_H - 1))
                        nc.vector.scalar_tensor_tensor(
                            dst[:, mt, _chsl(ch)], ps[:],
                            bias_sb["bvo"][:, mt:mt + 1],
                            res[:, mt, _chsl(ch)], op0=ALU.add, op1=ALU.add)
                for x_bf, x2 in ((x0, x2a), (x1, x2b)):
                    for ch in range(CH):
                        nc.vector.tensor_mul(out=x2[:, mt, _chsl(ch)],
                                             in0=x_bf[:, mt, _chsl(ch)],
                                             in1=x_bf[:, mt, _chsl(ch)])
            cd_f8 = sp_tile("cd", dt=FP8)  # (c0-c1)/8, built per tile
            # interleave so the PE stat MMs stay contiguous (paux ring 2
            # frees early via the sm chains) while lnp ring 3 carries the
            # stats across the norm chains
            st_x0c0 = stats_mms(x0, x2a, 0)
            st_x0c1 = stats_mms(x0, x2a, 1)
            ln_norm(x0, st_x0c0, 0, "n1g", "n1b", c0, c0_f8)
            st_x1c0 = stats_mms(x1, x2b, 0)
            ln_norm(x0, st_x0c1, 1, "n1g", "n1b", c0, c0_f8)
            st_x1c1 = stats_mms(x1, x2b, 1)
            for ch, st in ((0, st_x1c0), (1, st_x1c1)):
                ln_norm(x1, st, ch, "n2g", "n2b", c1, c1_f8)
                for k in range(KT_H):
                    nc.vector.tensor_sub(out=cd_f8[:, k, _chsl(ch)],
                                         in0=c0_f8[:, k, _chsl(ch)],
                                         in1=c1_f8[:, k, _chsl(ch)])

            # ============ P3g: gate logits (independent filler) ==========
            # Pure GEMMs with ACT evictions: the PE stays busy here while
            # the LN chains drain on DVE; P4's weights prefetch at the tail.
            g_sb = act_tile("SG", "g_sb")
            sid3g, _ = nc.enter_named_scope("P3g", False)
            qkv_pre = None
            for mt in range(KT_H):
                wgi = load_w(gwiT, KT_H, mt, "wgi")
                wgt = load_w(gwtT, KT_H, mt, "wgt")
                for ch in range(CH):
                    ps = pmain.tile([P, CHS], F32, tag="mm", name="ps_g")
                    for k in range(KT_H):
                        nc.tensor.matmul(ps[:], lhsT=wgi[:, k, :],
                                         rhs=imgp[:, k, _chsl(ch)],
                                         start=(k == 0), stop=False)
                    for k in range(KT_H):
                        nc.tensor.matmul(ps[:], lhsT=wgt[:, k, :],
                                         rhs=txtp[:, k, _chsl(ch)],
                                         start=False, stop=(k == KT_H - 1))
                    # sigmoid fused into the eviction: g_sb holds the gate
                    nc.scalar.activation(g_sb[:, mt, _chsl(ch)], ps[:], AF.Sigmoid,
                                         bias=bias_sb["gb"][:, mt:mt + 1], scale=1.0)
                if mt == KT_H - 2:
                    # prefetch only P4's first-needed (score-path) weights;
                    # 5 DMAs here congested the queue and stalled P3g's tail
                    qkv_pre = [load_w8(w_d, KT_H, 0, nm) for w_d, nm in
                               ((wqT, "wq"), (wkT, "wk"), (wqbT, "wqb"))]
            nc.leave_named_scope("P3g", sid3g, False)

            # ========= P4: self-attention qkv + scores (fp8 DR) ==========
            v0 = act_tile("S5", "v0")   # v/8 (bf16); recycles x2a
            v1 = act_tile("S6", "v1")
            o0 = act_tile("S1", "o0", FP8)  # o/8, after imgp's last read
            o1 = act_tile("S2", "o1", FP8)
            pca_cm.__exit__(None, None, None)  # free x1's 16KB for pqk
            sid4, _ = nc.enter_named_scope("P4", False)
            pqk_cm, pqk = open_pool(name="pqk", bufs=1)

            def qkv8(wt, act8, bname, mt, dst_t, dst_mt=None, scale=1.0,
                     dve=False):
                for ch in range(CH):
                    ps = pmain.tile([P, CHS], F32, tag="mm", name="ps_qkv")
                    mm_dr(ps, wt, act8, KT_H, ch)
                    bias = bias_sb[bname][:, mt:mt + 1] if bname else zero_col[:, :]
                    dst = (dst_t[:, _chsl(ch)] if dst_mt is None
                           else dst_t[:, dst_mt, _chsl(ch)])
                    if dve:
                        # DVE eviction: ACT is the tighter engine in P4
                        nc.vector.tensor_scalar(dst, ps[:], scale, bias,
                                                op0=ALU.mult, op1=ALU.add)
                    else:
                        nc.scalar.activation(dst, ps[:], AF.Identity,
                                             bias=bias, scale=scale)

            hm2 = hmask_sb[:, :]    # [128, 2] local-head one-hot
            AB = float(SA / SB)  # hmaskT_sb[0:2] is the [2,128] broadcast mask

            def gemm_part(mt, wts):
                """GEMMs + the DVE muls that feed the score chain."""
                wq_t, wv_t, wk_t, wqb_t, wvb_t = wts
                q0t = pqk.tile([P, R], BF16, tag="q0t", bufs=2)
                q1t = pqk.tile([P, R], BF16, tag="q1t", bufs=2)
                kdt = pqk.tile([P, R], BF16, tag="kdt")
                qkv8(wq_t, c0_f8, "sbq", mt, q0t)
                qkv8(wk_t, cd_f8, None, mt, kdt)  # k0-k1; bias cancels
                qkv8(wqb_t, c1_f8, "sbq", mt, q1t, scale=AB)
                nc.vector.tensor_mul(out=q0t[:], in0=q0t[:], in1=kdt[:])
                nc.vector.tensor_mul(out=q1t[:], in0=q1t[:], in1=kdt[:])
                qkv8(wv_t, c0_f8, "sbv8", mt, v0, dst_mt=mt, scale=1.0 / 8.0)
                qkv8(wvb_t, c1_f8, "sbv8", mt, v1, dst_mt=mt, scale=1.0 / SB)
                return mt, q0t, q1t

            def tail_part(st):
                """Score matmuls + attention combine for a PREVIOUS mt:
                emitted one iteration late so the cross-engine latency
                chain (evict->mul->dmm->sigmoid->ab) hides under the next
                mt's GEMM stream instead of stalling the PE."""
                mt, m0, m1 = st
                a_ts = {}
                for ch in range(CH):
                    for m_t, nm in ((m0, "A"), (m1, "B")):
                        dmm = paux.tile([2, CHS], F32, tag="Sb", name=f"dmm{nm}")
                        nc.tensor.matmul(dmm[:], lhsT=hm2, rhs=m_t[:, _chsl(ch)],
                                         start=True, stop=True)
                        a_t = spool.tile([2, CHS], BF16, tag="small", name=f"a{nm}")
                        nc.scalar.activation(a_t[:], dmm[:], AF.Sigmoid,
                                             bias=zero_col[0:2, :],
                                             scale=float(1.0 / np.sqrt(HD)))
                        a_ts[(ch, nm)] = a_t
                for ch in range(CH):
                    diff = tpool.tile([P, CHS], BF16, tag="tmp", name="att_diff")
                    nc.vector.tensor_sub(out=diff[:], in0=v0[:, mt, _chsl(ch)],
                                         in1=v1[:, mt, _chsl(ch)])
                    for o_t, nm in ((o0, "A"), (o1, "B")):
                        ab = paux.tile([P, CHS], F32, tag="Qb", name=f"ab{nm}")
                        nc.tensor.matmul(ab[:], lhsT=hmaskT_sb[0:2, :],
                                         rhs=a_ts[(ch, nm)][:],
                                         start=True, stop=True)
                        t_t = tpool.tile([P, CHS], BF16, tag="tmp", name=f"att_t{nm}")
                        nc.vector.tensor_mul(out=t_t[:], in0=diff[:], in1=ab[:])
                        nc.vector.tensor_add(out=o_t[:, mt, _chsl(ch)], in0=t_t[:],
                                             in1=v1[:, mt, _chsl(ch)])

            wq_next = None
            pend = None
            for mt in range(KT_H):
                if mt == 0:
                    wq0, wk0, wqb0 = qkv_pre
                    wts = (wq0,
                           load_w8(wvT, KT_H, 0, "wv"),
                           wk0, wqb0,
                           load_w8(wvbT, KT_H, 0, "wvb"))
                else:
                    wts = (wq_next,
                           load_w8(wvT, KT_H, mt, "wv"),
                           load_w8(wkT, KT_H, mt, "wk"),
                           load_w8(wqbT, KT_H, mt, "wqb"),
                           load_w8(wvbT, KT_H, mt, "wvb"))
                st = gemm_part(mt, wts)
                if mt + 1 < KT_H:
                    wq_next = load_w8(wqT, KT_H, mt + 1, "wq")
                if pend is not None:
                    tail_part(pend)
                pend = st
            tail_part(pend)
            # prefetch P5's first wo tile into the w_h ring
            wo_pre = load_w8(woT, KT_H, 0, "wo")
            nc.leave_named_scope("P4", sid4, False)

            # prefetch the first two FFN w1 tiles into the big-weight slots
            w1_pre = []
            for mt in range(2):
                t = wpool.tile([P, KT_F, P], FP8, tag="w", name=f"w1_pre{mt}")
                nc.sync.dma_start(t[:, :KT_H, :], w1aT[:, mt])
                w1_pre.append(t)

            # ===== P5: SA out-proj (fp8 DR) + residual + LN3 =====
            # Residual folded into PSUM via an identity matmul; ACT evicts.
            # LN3's bf16 output is stored as r + ffn_b2 (n3bf = n3_b + fb2)
            # so the FFN w2 evict needs no extra bias op.
            r0 = act_tile("S1", "r0")    # r0 + fb2 (bf16); reuses o0 slot
            r1 = act_tile("S2", "r1")
            r0_f8 = act_tile("C8a", "r0_f8", FP8, pad16=False)
            r1_f8 = act_tile("C8b", "r1_f8", FP8, pad16=False)
            imgp2 = act_tile("S3", "imgp2")
            txtp2 = act_tile("S4", "txtp2")
            pqk_cm.__exit__(None, None, None)  # free 10KB for psa
            sid5, _ = nc.enter_named_scope("P5", False)
            psa_cm, psa = open_pool(name="psa", bufs=1)
            y0 = sp_tile("y0")
            y1 = psa.tile([P, KT_H, R], BF16, tag="y1", name="y1")
            y2a = act_tile("S5", "y2a")  # y0^2; recycles v0 slot
            y2b = act_tile("S6", "y2b")

            # y0's whole pipeline runs FIRST so its stats/norm chain (the
            # FFN pos0 critical path) starts ~17us earlier; y1's GEMMs then
            # cover the y0 norm DVE work. wo tiles are loaded twice (+1MB).
            def wo_pass(o_t, res, dst, y2, first_pass):
                for mt in range(KT_H):
                    if first_pass and mt == 0:
                        wt = wo_pre
                    else:
                        wt = load_w8(woT, KT_H, mt, "wo")
                    for ch in range(CH):
                        ps = pmain.tile([P, CHS], F32, tag="mm", name="ps_wo")
                        mm_dr(ps, wt, o_t, KT_H, ch, stop=False)
                        nc.tensor.matmul(ps[:], lhsT=ident_sb[:],
                                         rhs=res[:, mt, _chsl(ch)],
                                         start=False, stop=True)
                        nc.scalar.activation(dst[:, mt, _chsl(ch)], ps[:],
                                             AF.Identity,
                                             bias=bias_sb["sbo"][:, mt:mt + 1],
                                             scale=1.0)
                    for ch in range(CH):
                        nc.vector.tensor_mul(out=y2[:, mt, _chsl(ch)],
                                             in0=dst[:, mt, _chsl(ch)],
                                             in1=dst[:, mt, _chsl(ch)])

            wo_pass(o0, c0, y0, y2a, True)
            for ch in range(CH):
                sty = stats_mms(y0, y2a, ch)
                ln_norm_raw(y0, sty, ch, r0, r0_f8)
            wo_pass(o1, c1, y1, y2b, False)
            for ch in range(CH):
                ln_gain_inplace(r0, ch, "n3gf", "n3bf")
            sty1 = [stats_mms(y1, y2b, ch) for ch in range(CH)]
            for ch in range(CH):
                ln_norm_raw(y1, sty1[ch], ch, r1, r1_f8, f8_dve=True)
                ln_gain_inplace(r1, ch, "n3gf", "n3bf")

            # ===== P6/P7: FFN both positions (fp8 DR); pooled accumulation
            # SAME pool scope as P5: a separate scope would barrier the FFN
            # start on the y1-stats PSUM retirement (9.5us measured).
            # mt-outer / ch-inner: each weight tile loaded once per position.
            pooled = sp_tile("pooled")  # bf16; pos1 fuses the final combine
            sid67, _ = nc.enter_named_scope("P67", False)
            gate_done = False
            for pos, (r_f8, r_p, w1d, w2d, first) in enumerate([
                    (r0_f8, r0, w1aT, w2aT, True),
                    (r1_f8, r1, w1bT, w2bT, False)]):
                if not first and not gate_done:
                    # g_sb <- gate*(imgp-txtp) + txtp in place; runs on DVE
                    # during pos0's PE work so the pos1 chain is 3 ops + DMA
                    gate_done = True
                    for gmt in range(KT_H):
                        for gch in range(CH):
                            gsl = _chsl(gch)
                            gd = tpool.tile([P, CHS], BF16, tag="tmp", name="gd")
                            nc.vector.tensor_sub(out=gd[:],
                                                 in0=imgp2[:, gmt, gsl],
                                                 in1=txtp2[:, gmt, gsl])
                            gt = tpool.tile([P, CHS], BF16, tag="tmp", name="gt")
                            nc.vector.tensor_mul(out=gt[:],
                                                 in0=g_sb[:, gmt, gsl], in1=gd[:])
                            nc.vector.tensor_add(out=g_sb[:, gmt, gsl],
                                                 in0=gt[:], in1=txtp2[:, gmt, gsl])
                            # fold the gate term into pooled here (DVE has
                            # slack under pos1's w1 GEMMs): shortens the
                            # per-(mt,ch) output chain to 2 ops
                            nc.vector.tensor_add(out=pooled[:, gmt, gsl],
                                                 in0=pooled[:, gmt, gsl],
                                                 in1=g_sb[:, gmt, gsl])
                gelu_scale = 1.0 if first else float(SA / SB)
                ev_scale = (1.0 / SA) if first else (1.0 / SB)
                # hidden [128, 16, 1024] fp8 x2 in the freed y2a/y2b slots
                h_a = acts.tile([P, KT_F // 2, R], FP8, tag="S5",
                                name=f"h_a{pos}")
                h_b = acts.tile([P, KT_F // 2, R], FP8, tag="S6",
                                name=f"h_b{pos}")
                if first:
                    # start with ch0-only chains: r0_f8[ch1] finishes on DVE
                    # a few us after ch0, so ch1 work is deferred past it
                    # (mt0/1 sit in the big w slots, so all 8 fit the ring)
                    NCH0 = 8
                    w1_order = ([(mt, 0) for mt in range(NCH0)]
                                + [(mt, 1) for mt in range(NCH0)]
                                + [(mt, ch) for mt in range(NCH0, KT_F)
                                   for ch in range(CH)])
                else:
                    w1_order = [(mt, ch) for mt in range(KT_F)
                                for ch in range(CH)]
                w1_tiles = {}
                for mt, ch in w1_order:
                    if mt not in w1_tiles:
                        if first and mt < 2:
                            w1_tiles[mt] = w1_pre[mt]
                        else:
                            w1_tiles[mt] = load_w8(w1d, KT_H, mt, "w1")
                    if first and ch == 0:
                        # reload the P8 gate operands (4MB) here, past the
                        # FFN head, so they don't contend with the w1 stream
                        if mt == 10:
                            nc.sync.dma_start(imgp2[:], imgp_d[:, :, :])
                        elif mt == 20:
                            nc.sync.dma_start(txtp2[:], txtp_d[:, :, :])
                    wt = w1_tiles[mt]
                    hdst = h_a if mt < KT_F // 2 else h_b
                    ps = pmain.tile([P, CHS], F32, tag="mm", name="ps_f1")
                    mm_dr(ps, wt, r_f8, KT_H, ch)
                    nc.scalar.activation(
                        hdst[:, mt % (KT_F // 2), _chsl(ch)], ps[:],
                        AF.Gelu, bias=fb1_sb[:, mt:mt + 1], scale=gelu_scale)
                for mt in range(KT_H):
                    wt = load_w8(w2d, KT_F, mt, "w2")
                    for ch in range(CH):
                        ps = pmain.tile([P, CHS], F32, tag="mm", name="ps_f2")
                        for k in range(KT_F // 4):
                            nc.tensor.matmul(
                                ps[:], lhsT=wt[:, 2 * k:2 * k + 2, :],
                                rhs=h_a[:, 2 * k:2 * k + 2, _chsl(ch)],
                                start=(k == 0), stop=False, perf_mode=DR)
                        for k in range(KT_F // 4):
                            nc.tensor.matmul(
                                ps[:],
                                lhsT=wt[:, KT_F // 2 + 2 * k:KT_F // 2 + 2 * k + 2, :],
                                rhs=h_b[:, 2 * k:2 * k + 2, _chsl(ch)],
                                start=False, stop=(k == KT_F // 4 - 1),
                                perf_mode=DR)
                        if first:
                            # r0 holds 0.5*(r+fb2): pooled = 0.5*p0
                            nc.vector.scalar_tensor_tensor(
                                pooled[:, mt, _chsl(ch)], ps[:], 0.5 * ev_scale,
                                r_p[:, mt, _chsl(ch)], op0=ALU.mult, op1=ALU.add)
                        else:
                            # final combine (gate pre-added into pooled):
                            # out = [0.5*p1] + [0.5*p0 + gate-term]
                            tmp = tpool.tile([P, CHS], F32, tag="tmp", name="ffn_tmp")
                            nc.vector.scalar_tensor_tensor(
                                tmp[:], ps[:], 0.5 * ev_scale,
                                r_p[:, mt, _chsl(ch)], op0=ALU.mult, op1=ALU.add)
                            fin = tpool.tile([P, CHS], F32, tag="tmp", name="gfin")
                            nc.vector.tensor_add(out=fin[:], in0=tmp[:],
                                                 in1=pooled[:, mt, _chsl(ch)])
                            nc.sync.dma_start(outT[mt * P:(mt + 1) * P, _chsl(ch)],
                                              fin[:])
            nc.leave_named_scope("P67", sid67, False)
            nc.leave_named_scope("P5", sid5, False)
            psa_cm.__exit__(None, None, None)

        lnp_cm.__exit__(None, None, None)
        acts_cm.__exit__(None, None, None)
        spool_cm.__exit__(None, None, None)
        tpool_cm.__exit__(None, None, None)
        wpool_cm.__exit__(None, None, None)
        const_cm.__exit__(None, None, None)

    nc.compile()
    return nc


def host_prep(inputs):
    """Host-side preprocessing: merge CA weights, transpose, cast, shard."""
    f = {k: np.asarray(v, dtype=np.float32) for k, v in inputs.items()}

    def bf(x):
        return np.ascontiguousarray(x).astype(np_bf16)

    def tile4(wT, np_dt):
        """[K, M] (already transposed) -> [P, MT, KT, P] pre-tiled layout."""
        K, M = wT.shape
        kt, mtn = K // P, M // P
        w4 = wT.reshape(kt, P, mtn, P).transpose(1, 2, 0, 3)
        return np.ascontiguousarray(w4).astype(np_dt)

    def q8t(w, s):
        """quantize w.T at scale s, pre-tiled."""
        return tile4(np.asarray(w, np.float32).T * s, np_fp8)

    def bft(w):
        return tile4(np.asarray(w, np.float32).T, np_bf16)

    def bias128(x, kt):
        return np.ascontiguousarray(np.asarray(x, np.float32).reshape(kt, P).T)

    ca_wv = np.split(f["ca_wqkv"], 3, axis=0)[2]
    ca_bv = f["ca_bqkv"][2 * H:]
    w_vo = f["ca_wo"] @ ca_wv
    b_vo = f["ca_wo"] @ ca_bv + f["ca_bo"]

    sa_wq, sa_wk, sa_wv = np.split(f["sa_wqkv"], 3, axis=0)
    sa_bq, sa_bk, sa_bv = np.split(f["sa_bqkv"], 3)

    gwi = f["gate_w"][:, :H]
    gwt = f["gate_w"][:, H:]

    # LN3 gain/bias fold for the FFN first layer (see kernel docstring)
    w1f = f["ffn_w1"] * f["n3_g"][None, :]
    fb1f = f["ffn_b1"] + f["ffn_w1"] @ f["n3_b"]

    lh = np.arange(P) // HD  # local head index within a 128-feature tile
    hmask = np.ascontiguousarray((lh[:, None] == np.arange(2)[None, :]).astype(np_bf16))
    hmaskT = np.zeros((34, P), np_bf16)  # copies at rows 0-1 and 32-33
    hmaskT[0:2] = hmask.T
    hmaskT[32:34] = hmask.T
    hmaskT = np.ascontiguousarray(hmaskT)

    shared = {
        "wiT": bf(f["Wi"].T), "wtT": bf(f["Wt"].T),
        "wvoT": bft(w_vo),
        "wqT": q8t(sa_wq, SA), "wqbT": q8t(sa_wq, SB),
        "wkT": q8t(sa_wk, SA),
        "wvT": q8t(sa_wv, SA), "wvbT": q8t(sa_wv, SB),
        "woT": q8t(f["sa_wo"], SA),
        # LN3 gain/bias folded into w1 (input-column scale) and its bias:
        # gelu(u@(w1*g).T + b1 + w1@b) == gelu((u*g+b)@w1.T + b1)
        "w1aT": q8t(w1f, SA), "w1bT": q8t(w1f, SB),
        "w2aT": q8t(f["ffn_w2"], SA), "w2bT": q8t(f["ffn_w2"], SB),
        "gwiT": bft(gwi), "gwtT": bft(gwt),
        "ident128": np.ascontiguousarray(np.eye(P, dtype=np_bf16)),
        "bias_all": np.concatenate([
            bias128(f["bi"], KT_H), bias128(f["bt"], KT_H), bias128(b_vo, KT_H),
            bias128(sa_bq, KT_H), bias128(sa_bv / 8.0, KT_H),
            bias128(f["sa_bo"], KT_H), bias128(f["ffn_b2"], KT_H),
            bias128(f["gate_b"], KT_H),
            bias128(f["n1_g"], KT_H), bias128(f["n1_b"], KT_H),
            bias128(f["n2_g"], KT_H), bias128(f["n2_b"], KT_H),
            bias128(0.5 * f["n3_g"], KT_H),
            bias128(0.5 * (f["n3_b"] + f["ffn_b2"]), KT_H),
            bias128(-f["ffn_b2"] / 8.0, KT_H),
            bias128(fb1f, KT_F),
        ], axis=1),
        "hmask": np.ascontiguousarray(hmask), "hmaskT": hmaskT,
    }

    xiT = f["image_features"].T.astype(np_bf16)  # [IMG_D, B]
    xtT = f["text_features"].T.astype(np_bf16)
    in_maps = []
    for c in range(N_CORES):
        m = dict(shared)
        m["xiT"] = np.ascontiguousarray(xiT[:, c * R:(c + 1) * R])
        m["xtT"] = np.ascontiguousarray(xtT[:, c * R:(c + 1) * R])
        in_maps.append(m)
    return in_maps


_NC_CACHE = None


def kernel(**inputs) -> np.ndarray:
    global _NC_CACHE
    if _NC_CACHE is None:
        _NC_CACHE = build()
    nc = _NC_CACHE
    in_maps = host_prep(inputs)
    res = run_bass_kernel_spmd(nc, in_maps, core_ids=list(range(N_CORES)))
    out = np.empty((B, H), np.float32)
    for c in range(N_CORES):
        out[c * R:(c + 1) * R, :] = res.results[c]["outT"].T
    return out


if __name__ == "__main__":
    nc = build()
    print("built OK")


# revision 64
# speedup vs baseline: 1.0056x; 1.0031x over previous
"""Trainium2 Bass kernel for nn_AttentionFusion (dense transformer block).

Data-parallel over batch: B=8192 rows sharded as 1024 rows per NeuronCore
across 8 cores; weights replicated. On-chip layout is feature-major:
activations are stored as [128 partitions(features), k_tiles, 1024 rows],
so every matmul is out.T[m,n] = sum_k W.T[k,m] * act.T[k,n] with natural
(host-pre-transposed) weight loads and the contraction on the partition dim.

Algebraic simplifications (validated against the reference to 2e-6):
  - Cross-attention has seq len 1 -> softmax == 1 -> out = v @ wo.T + bo;
    additionally (v @ wv.T) @ wo.T = v @ (wo@wv).T is merged on the host.
  - Self-attention has seq len 2 -> softmax([a,b]) = [sig(a-b), 1-sig(a-b)].
  - LayerNorm / attention-score reductions over features (= partitions) are
    done with small matmuls against ones/head-mask matrices.

fp8 (e4m3) DoubleRow matmuls (2x contraction per instruction):
  - SA q/k/v, SA out-proj, FFN w1/w2 run as fp8 DoubleRow.
  - Weights host-quantized at 8x scale (12x for the second FFN position to
    decorrelate quantization noise between the two pooled positions);
    activations stored as value/8 in fp8 so PSUM results are at true scale.
  - Cross-attention / gate / input projections stay bf16 (noise budget).

Scheduling (655us -> 610us measured): every matmul streams 512 columns
(~216 ns) regardless of dtype (DoubleRow's 2x is the doubled contraction
per instruction), so the wins are (a) keeping the in-order PE queue free
of instructions that wait on DVE/ACT, and (b) never closing a PSUM pool
mid-kernel:
  - weights host-pre-tiled [P, MT, KT, 128]: each weight-tile DMA is one
    contiguous per-partition run (no 128B-chunk gathers).
  - ONE PSUM pool scope spans P23..P67 (a scope boundary barriers the next
    phase's first matmul on the previous phase's PSUM retirement).
  - P23: x0/x1 GEMMs share one wvo load per mt; DVE evicts (bias+residual);
    x^2 staged into the free S5/S6 slots; stat MMs emitted after the GEMMs
    (interleaved so lnp ring 2 suffices); the independent gate GEMMs (P3g,
    sigmoid fused into the ACT eviction) cover the LN chains on DVE.
  - LN fp8 copies derived from the bf16 value on ACT (out/8); for LN3 the
    gain/bias are folded into w1/b1 host-side so the FFN consumes the raw
    normalized value (gain applied later, in place, off the critical path).
  - P4 software-pipelined by one mt: the score chain (evict->mul->dmm->
    sigmoid->ab) of mt runs under mt+1's GEMM stream.
  - P5: residual-add folded into PSUM via an identity matmul, ACT evicts;
    y0's whole pipeline (GEMMs->stats->norm) runs before y1's GEMMs so
    r0_f8 is ready when the FFN starts (which begins ch0-only);
    LN3-y1 and the imgp/txtp reload hide under FFN pos0.
  - output chain: r bf16 copies stored pre-halved and the gate term
    pre-added into pooled, so each pos1 eviction is 2 DVE ops + DMA.
"""

import numpy as np
import ml_dtypes

import concourse.bacc as bacc
import concourse.mybir as mybir
import concourse.tile as tile
from concourse.bass_utils import run_bass_kernel_spmd

AF = mybir.ActivationFunctionType
ALU = mybir.AluOpType
BF16 = mybir.dt.bfloat16
F32 = mybir.dt.float32
FP8 = mybir.dt.float8e4
DR = mybir.MatmulPerfMode.DoubleRow

N_CORES = 8
B, IMG_D, TXT_D, H, NH = 8192, 1280, 2048, 1024, 16
HD = H // NH  # 64 head dim
R = B // N_CORES  # 1024 rows per core
P = 128
CH = 2  # row chunks per core
CHS = R // CH  # 512 rows per chunk
KT_I, KT_T, KT_H, KT_F = IMG_D // P, TXT_D // P, H // P, 4 * H // P
EPS = 1e-5
SA = 8.0    # fp8 weight scale, position 0 / shared
SB = 12.0   # fp8 weight scale, position 1 (FFN dual-quantization)

np_bf16 = ml_dtypes.bfloat16
np_fp8 = ml_dtypes.float8_e4m3

def _chsl(ch):
    return slice(ch * CHS, (ch + 1) * CHS)


def build():
    nc = bacc.Bacc(None, target_bir_lowering=False)

    def din(name, shape, dt=BF16):
        return nc.dram_tensor(name, shape, dt, kind="ExternalInput")

    xiT = din("xiT", [IMG_D, R])
    xtT = din("xtT", [TXT_D, R])
    wiT = din("wiT", [IMG_D, H])
    wtT = din("wtT", [TXT_D, H])
    # pre-tiled weights: [P, MT, KT, 128]; slice [:, mt] is one SBUF tile,
    # contiguous per partition (KT*128 elements)
    wvoT = din("wvoT", [P, KT_H, KT_H, P])
    wqT = din("wqT", [P, KT_H, KT_H, P], FP8)
    wqbT = din("wqbT", [P, KT_H, KT_H, P], FP8)
    wkT = din("wkT", [P, KT_H, KT_H, P], FP8)
    wvT = din("wvT", [P, KT_H, KT_H, P], FP8)
    wvbT = din("wvbT", [P, KT_H, KT_H, P], FP8)
    woT = din("woT", [P, KT_H, KT_H, P], FP8)
    w1aT = din("w1aT", [P, KT_F, KT_H, P], FP8)
    w1bT = din("w1bT", [P, KT_F, KT_H, P], FP8)
    w2aT = din("w2aT", [P, KT_H, KT_F, P], FP8)
    w2bT = din("w2bT", [P, KT_H, KT_F, P], FP8)
    gwiT = din("gwiT", [P, KT_H, KT_H, P])
    gwtT = din("gwtT", [P, KT_H, KT_H, P])
    ident_d = din("ident128", [P, P])

    bias_names = ["bi", "bt", "bvo", "sbq", "sbv8", "sbo", "fb2", "gb",
                  "n1g", "n1b", "n2g", "n2b", "n3gf", "n3bf",
                  "fb2n8"]
    NB = len(bias_names) * KT_H + KT_F
    bias_all_d = din("bias_all", [P, NB], F32)
    hmask_d = din("hmask", [P, 2])
    hmaskT_d = din("hmaskT", [34, P])  # rows 0-1 and 32-33 hold hmask.T

    # DRAM spill for imgp/txtp between P3 and P8 (frees SBUF during SA/FFN)
    imgp_d = nc.dram_tensor("imgp_spill", [P, KT_H, R], BF16)
    txtp_d = nc.dram_tensor("txtp_spill", [P, KT_H, R], BF16)

    outT = nc.dram_tensor("outT", [H, R], F32, kind="ExternalOutput")

    with tile.TileContext(nc) as tc:
        def open_pool(**kw):
            cm = tc.tile_pool(**kw)
            return cm, cm.__enter__()

        def scope(name):
            import contextlib

            @contextlib.contextmanager
            def _s():
                sid, _ = nc.enter_named_scope(name, False)
                yield
                nc.leave_named_scope(name, sid, False)
            return _s()

        # -------- constants (whole kernel) --------
        const_cm, const = open_pool(name="const", bufs=1)
        ones128 = const.tile([P, P], BF16)
        nc.vector.memset(ones128[:], 1.0)
        eps_col = const.tile([P, 1], F32)
        nc.vector.memset(eps_col[:], EPS)
        zero_col = const.tile([P, 1], F32)
        nc.vector.memset(zero_col[:], 0.0)
        bias_all = const.tile([P, NB], F32)
        bias_sb = {n: bias_all[:, i * KT_H:(i + 1) * KT_H]
                   for i, n in enumerate(bias_names)}
        fb1_sb = bias_all[:, len(bias_names) * KT_H:]
        hmask_sb = const.tile([P, 2], BF16)
        hmaskT_sb = const.tile([34, P], BF16)
        ident_sb = const.tile([P, P], BF16)
        ones2_f8 = const.tile([P, 2, P], FP8)  # DR ones for fp8 LN stats
        nc.vector.memset(ones2_f8[:], 1.0)

        def load_consts():
            nc.sync.dma_start(bias_all[:], bias_all_d[:, :])
            nc.sync.dma_start(hmask_sb[:], hmask_d[:, :])
            nc.sync.dma_start(hmaskT_sb[:], hmaskT_d[:, :])
            nc.sync.dma_start(ident_sb[:], ident_d[:, :])

        # -------- shared SBUF pools (whole kernel) --------
        wpool_cm, wpool = open_pool(name="wpool", bufs=2)   # "w" 4KB slots x2
        tpool_cm, tpool = open_pool(name="tpool", bufs=6)   # "tmp" 2KB x6
        spool_cm, spool = open_pool(name="spool", bufs=4)   # "small" 2KB x4
        acts_cm, acts = open_pool(name="acts", bufs=1)

        def act_tile(tag, name, dt=BF16, pad16=True):
            shape = [P, KT_H, R]
            pad = None
            if dt == FP8 and pad16:
                pad = [P, KT_H, 2 * R]  # keep the recycled slot at 16KB
            return acts.tile(shape, dt, tag=tag, name=name, padded_shape=pad)

        def sp_tile(name, shape=None, dt=BF16):
            """SP slot is 16KB/partition (bf16 [P, KT_H, R])."""
            shape = shape or [P, KT_H, R]
            pad = None
            if mybir.dt.size(dt) == 1:
                pad = [shape[0], shape[1], shape[2] * 2]
            return acts.tile(shape, dt, tag="SP", name=name, padded_shape=pad)

        pmain = None
        paux = None

        def load_w(w4_d, kt, mt, name):
            """bf16 weight tile [128, kt, 128] from pre-tiled DRAM [:, mt]."""
            if kt > KT_H:
                t = wpool.tile([P, KT_F, P], BF16, tag="w", name=name)
            else:
                t = wpool.tile([P, KT_H, P], BF16, tag="w_h", name=name, bufs=7)
            nc.sync.dma_start(t[:, :kt, :], w4_d[:, mt])
            return t

        def load_w8(w4_d, kt, mt, name):
            """fp8 weight tile [128, kt, 128] (big 'w' slot is 4KB fp8)."""
            if kt > KT_H:
                t = wpool.tile([P, KT_F, P], FP8, tag="w", name=name)
            else:
                t = wpool.tile([P, KT_H, P], FP8, tag="w_h", name=name, bufs=7,
                               padded_shape=[P, KT_H, 2 * P])
            nc.sync.dma_start(t[:, :kt, :], w4_d[:, mt])
            return t

        def mm_dr(ps, wt, act8, kt, ch, start=True, stop=True):
            """fp8 DoubleRow accumulation chain: kt k-tiles as kt//2 pairs."""
            np_ = kt // 2
            for k in range(np_):
                nc.tensor.matmul(
                    ps[:], lhsT=wt[:, 2 * k:2 * k + 2, :],
                    rhs=act8[:, 2 * k:2 * k + 2, _chsl(ch)],
                    start=(start and k == 0), stop=(stop and k == np_ - 1),
                    perf_mode=DR,
                )

        lnp_cm, lnp = open_pool(name="lnp", bufs=2)  # LN stats (mf bf16, ivf f32)

        def _stats_finish(sb_, qb_, s_scale, q_scale):
            mf = lnp.tile([P, CHS], BF16, tag="lnm", name="ln_mf")
            nc.vector.tensor_scalar_mul(mf[:], sb_[:], s_scale)
            msq = tpool.tile([P, CHS], F32, tag="tmp", name="ln_msq")
            nc.vector.tensor_mul(out=msq[:], in0=mf[:], in1=mf[:])
            vf = tpool.tile([P, CHS], F32, tag="tmp", name="ln_vf")
            nc.vector.scalar_tensor_tensor(vf[:], qb_[:], q_scale, msq[:],
                                           op0=ALU.mult, op1=ALU.subtract)
            sd = tpool.tile([P, CHS], F32, tag="tmp", name="ln_sd")
            nc.scalar.activation(sd[:], vf[:], AF.Sqrt, bias=eps_col[:], scale=1.0)
            # ~5x faster than nc.vector.reciprocal (which stalled PE 3.4us)
            ivf_f = tpool.tile([P, CHS], F32, tag="tmp", name="ln_ivf_f")
            nc.vector.reciprocal_approx_fast(out=ivf_f[:], in_=sd[:])
            ivf = lnp.tile([P, CHS], BF16, tag="lni", name="ln_ivf")
            nc.vector.tensor_scalar_mul(ivf[:], ivf_f[:], 1.0)
            return mf, ivf

        def stats_mms(x_bf, x2, ch):
            """LN row mean + rsqrt(var): Sb/Qb ones-matmuls over pre-staged
            x^2 tiles, then the small DVE/ACT finishing chain."""
            sb_ = paux.tile([P, CHS], F32, tag="Sb", name="ln_Sb")
            for k in range(KT_H):
                nc.tensor.matmul(sb_[:], lhsT=ones128[:],
                                 rhs=x_bf[:, k, _chsl(ch)],
                                 start=(k == 0), stop=(k == KT_H - 1))
            qb_ = paux.tile([P, CHS], F32, tag="Qb", name="ln_Qb")
            for k in range(KT_H):
                nc.tensor.matmul(qb_[:], lhsT=ones128[:],
                                 rhs=x2[:, k, _chsl(ch)],
                                 start=(k == 0), stop=(k == KT_H - 1))
            return _stats_finish(sb_, qb_, 1.0 / H, 1.0 / H)

        def stats_mms_dr(xp, ch):
            """fp8 DoubleRow LN stats: xp packs fp8(x) at [:, :, 0:R] and
            fp8(x)^2 at [:, :, R:2R]; half the matmul slots of stats_mms.
            Stat quantization noise is ~0.1% of the LN scale (negligible)."""
            sb_ = paux.tile([P, CHS], F32, tag="Sb", name="ln_Sb8")
            for k in range(KT_H // 2):
                nc.tensor.matmul(sb_[:], lhsT=ones2_f8[:, 0:2, :],
                                 rhs=xp[:, 2 * k:2 * k + 2, ch * CHS:(ch + 1) * CHS],
                                 start=(k == 0), stop=(k == KT_H // 2 - 1),
                                 perf_mode=DR)
            qb_ = paux.tile([P, CHS], F32, tag="Qb", name="ln_Qb8")
            for k in range(KT_H // 2):
                nc.tensor.matmul(qb_[:], lhsT=ones2_f8[:, 0:2, :],
                                 rhs=xp[:, 2 * k:2 * k + 2,
                                        R + ch * CHS:R + (ch + 1) * CHS],
                                 start=(k == 0), stop=(k == KT_H // 2 - 1),
                                 perf_mode=DR)
            return _stats_finish(sb_, qb_, 1.0 / H, 1.0 / H)

        def ln_norm(x_bf, stats, ch, g_name, b_name, out_bf,
                    out_f8=None):
            """Normalize: 3 DVE ops per k-tile; the fp8 copy (= out_bf/8)
            is derived on ACT."""
            g = bias_sb[g_name]
            bb = bias_sb[b_name]
            mf, ivf = stats
            for k in range(KT_H):
                t1 = tpool.tile([P, CHS], BF16, tag="tmp", name="ln_t1")
                nc.vector.tensor_sub(out=t1[:], in0=x_bf[:, k, _chsl(ch)], in1=mf[:])
                t2 = tpool.tile([P, CHS], BF16, tag="tmp", name="ln_t2")
                nc.vector.tensor_mul(out=t2[:], in0=t1[:], in1=ivf[:])
                nc.vector.tensor_scalar(out_bf[:, k, _chsl(ch)], t2[:],
                                        g[:, k:k + 1], bb[:, k:k + 1],
                                        op0=ALU.mult, op1=ALU.add)
                if out_f8 is not None:
                    nc.scalar.activation(
                        out_f8[:, k, _chsl(ch)], out_bf[:, k, _chsl(ch)],
                        AF.Identity, bias=zero_col[:, :], scale=1.0 / 8.0)

        def ln_norm_raw(x_bf, stats, ch, out_bf, out_f8, f8_dve=False):
            """FFN-path normalize: writes the RAW normalized value u into
            out_bf and u/8 into out_f8 (the LN gain/bias are folded into the
            FFN w1 weights host-side). The gain/bias for the bf16 residual
            copy are applied later, in place, off the critical path.
            f8_dve routes the fp8 copy to DVE (keeps ACT free for the FFN
            gelu evictions that run concurrently)."""
            mf, ivf = stats
            for k in range(KT_H):
                t1 = tpool.tile([P, CHS], BF16, tag="tmp", name="ln_t1")
                nc.vector.tensor_sub(out=t1[:], in0=x_bf[:, k, _chsl(ch)], in1=mf[:])
                nc.vector.tensor_mul(out=out_bf[:, k, _chsl(ch)], in0=t1[:],
                                     in1=ivf[:])
                if f8_dve:
                    nc.vector.tensor_scalar_mul(
                        out_f8[:, k, _chsl(ch)], out_bf[:, k, _chsl(ch)],
                        1.0 / 8.0)
                else:
                    nc.scalar.activation(
                        out_f8[:, k, _chsl(ch)], out_bf[:, k, _chsl(ch)],
                        AF.Identity, bias=zero_col[:, :], scale=1.0 / 8.0)

        def ln_gain_inplace(out_bf, ch, g_name, b_name):
            """Deferred: out_bf = out_bf * g + b, in place (DVE)."""
            g = bias_sb[g_name]
            bb = bias_sb[b_name]
            for k in range(KT_H):
                nc.vector.tensor_scalar(out_bf[:, k, _chsl(ch)],
                                        out_bf[:, k, _chsl(ch)],
                                        g[:, k:k + 1], bb[:, k:k + 1],
                                        op0=ALU.mult, op1=ALU.add)

        # ================= P0/P1: input projections (streamed) =============
        imgp = act_tile("S1", "imgp")
        txtp = act_tile("S2", "txtp")

        def input_proj(xT_d, w_d, kt_in, bname, dst, post_dma=None):
            for ch in range(CH):
                pss = [pmain.tile([P, CHS], F32, tag=f"mm{mt}", name=f"ps{mt}")
                       for mt in range(KT_H)]
                for k in range(kt_in):
                    wt = wpool.tile([P, H], BF16, tag="w_h", name="wrow", bufs=7)
                    nc.sync.dma_start(wt[:], w_d[k * P:(k + 1) * P, :])
                    xs = tpool.tile([P, CHS], BF16, tag="tmp", name="xslice")
                    nc.sync.dma_start(xs[:], xT_d[k * P:(k + 1) * P, _chsl(ch)])
                    for mt in range(KT_H):
                        nc.tensor.matmul(pss[mt][:], lhsT=wt[:, mt * P:(mt + 1) * P],
                                         rhs=xs[:], start=(k == 0), stop=(k == kt_in - 1))
                if post_dma is not None:
                    post_dma()
                    post_dma = None
                # alternate eviction engines so the tail of the last chunk
                # drains in half the time (frees PSUM banks for P23 sooner)
                for mt in range(KT_H):
                    if mt % 2 == 0:
                        nc.scalar.activation(dst[:, mt, _chsl(ch)], pss[mt][:],
                                             AF.Identity,
                                             bias=bias_sb[bname][:, mt:mt + 1],
                                             scale=1.0)
                    else:
                        nc.vector.tensor_scalar(dst[:, mt, _chsl(ch)], pss[mt][:],
                                                bias_sb[bname][:, mt:mt + 1], None,
                                                op0=ALU.add)

        with scope("P01"), tc.tile_pool(name="pmm01", bufs=1, space="PSUM") as pmain:
            input_proj(xiT, wiT, KT_I, "bi", imgp, post_dma=load_consts)
            input_proj(xtT, wtT, KT_T, "bt", txtp)
            nc.sync.dma_start(imgp_d[:, :, :], imgp[:])
            nc.sync.dma_start(txtp_d[:, :, :], txtp[:])
            # prefetch P23's first two weight tiles into the big-weight slots
            wvo_pre = []
            for mt in range(2):
                t = wpool.tile([P, KT_H, P], BF16, tag="w", name=f"wvo_pre{mt}",
                               padded_shape=[P, 2 * KT_H, P])
                nc.sync.dma_start(t[:, :, :], wvoT[:, mt])
                wvo_pre.append(t)

        # ============ P2/P3: merged cross-attention + LN ============
        c0 = act_tile("S3", "c0")
        c1 = act_tile("S4", "c1")
        c0_f8 = act_tile("C8a", "c0_f8", FP8, pad16=False)  # c0/8 for DR rhs
        c1_f8 = act_tile("C8b", "c1_f8", FP8, pad16=False)

        # ONE PSUM scope spans P23+P3g+P4: separate scopes would barrier
        # each phase's first matmul on the previous phase's PSUM retirement
        # (23.6us measured at P23->P3g). P4's score tiles overlay the
        # Sb/Qb stat tags.
        with (
            scope("P23"),
            tc.tile_pool(name="pmm234", bufs=4, space="PSUM") as pmain,
            tc.tile_pool(name="paux234", bufs=2, space="PSUM") as paux,
        ):
            pca_cm, pca = open_pool(name="pca", bufs=1)
            # x0/x1 GEMMs share one wvo load per mt; DVE evicts
            # (bias + residual); x^2 tiles staged into the free S5/S6 slots
            # right after each mt's evictions so the stat MMs never stall.
            x0 = sp_tile("x0")
            x1 = pca.tile([P, KT_H, R], BF16, tag="x1", name="x1")
            x2a = act_tile("S5", "x2a")  # x0^2
            x2b = act_tile("S6", "x2b")  # x1^2
            for mt in range(KT_H):
                wt = wvo_pre[mt] if mt < 2 else load_w(wvoT, KT_H, mt, "wvo")
                for src, res, dst in ((txtp, imgp, x0), (imgp, txtp, x1)):
                    for ch in range(CH):
                        ps = pmain.tile([P, CHS], F32, tag="mm", name="ps_mm")
                        for k in range(KT_H):
                            nc.tensor.matmul(ps[:], lhsT=wt[:, k, :],
                                             rhs=src[:, k, _chsl(ch)],
                                             start=(k == 0), stop=(k == KT_H - 1))
                        nc.vector.scalar_tensor_tensor(
                            dst[:, mt, _chsl(ch)], ps[:],
                            bias_sb["bvo"][:, mt:mt + 1],
                            res[:, mt, _chsl(ch)], op0=ALU.add, op1=ALU.add)
                for x_bf, x2 in ((x0, x2a), (x1, x2b)):
                    for ch in range(CH):
                        nc.vector.tensor_mul(out=x2[:, mt, _chsl(ch)],
                                             in0=x_bf[:, mt, _chsl(ch)],
                                             in1=x_bf[:, mt, _chsl(ch)])
            cd_f8 = sp_tile("cd", dt=FP8)  # (c0-c1)/8, built per tile
            # interleave so the PE stat MMs stay contiguous (paux ring 2
            # frees early via the sm chains) while lnp ring 3 carries the
            # stats across the norm chains
            st_x0c0 = stats_mms(x0, x2a, 0)
            st_x0c1 = stats_mms(x0, x2a, 1)
            ln_norm(x0, st_x0c0, 0, "n1g", "n1b", c0, c0_f8)
            st_x1c0 = stats_mms(x1, x2b, 0)
            ln_norm(x0, st_x0c1, 1, "n1g", "n1b", c0, c0_f8)
            st_x1c1 = stats_mms(x1, x2b, 1)
            for ch, st in ((0, st_x1c0), (1, st_x1c1)):
                ln_norm(x1, st, ch, "n2g", "n2b", c1, c1_f8)
                for k in range(KT_H):
                    nc.vector.tensor_sub(out=cd_f8[:, k, _chsl(ch)],
                                         in0=c0_f8[:, k, _chsl(ch)],
                                         in1=c1_f8[:, k, _chsl(ch)])

            # ============ P3g: gate logits (independent filler) ==========
            # Pure GEMMs with ACT evictions: the PE stays busy here while
            # the LN chains drain on DVE; P4's weights prefetch at the tail.
            g_sb = act_tile("SG", "g_sb")
            sid3g, _ = nc.enter_named_scope("P3g", False)
            qkv_pre = None
            for mt in range(KT_H):
                wgi = load_w(gwiT, KT_H, mt, "wgi")
                wgt = load_w(gwtT, KT_H, mt, "wgt")
                for ch in range(CH):
                    ps = pmain.tile([P, CHS], F32, tag="mm", name="ps_g")
                    for k in range(KT_H):
                        nc.tensor.matmul(ps[:], lhsT=wgi[:, k, :],
                                         rhs=imgp[:, k, _chsl(ch)],
                                         start=(k == 0), stop=False)
                    for k in range(KT_H):
                        nc.tensor.matmul(ps[:], lhsT=wgt[:, k, :],
                                         rhs=txtp[:, k, _chsl(ch)],
                                         start=False, stop=(k == KT_H - 1))
                    # sigmoid fused into the eviction: g_sb holds the gate
                    nc.scalar.activation(g_sb[:, mt, _chsl(ch)], ps[:], AF.Sigmoid,
                                         bias=bias_sb["gb"][:, mt:mt + 1], scale=1.0)
                if mt == KT_H - 2:
                    # prefetch only P4's first-needed (score-path) weights;
                    # 5 DMAs here congested the queue and stalled P3g's tail
                    qkv_pre = [load_w8(w_d, KT_H, 0, nm) for w_d, nm in
                               ((wqT, "wq"), (wkT, "wk"), (wqbT, "wqb"))]
            nc.leave_named_scope("P3g", sid3g, False)

            # ========= P4: self-attention qkv + scores (fp8 DR) ==========
            v0 = act_tile("S5", "v0")   # v/8 (bf16); recycles x2a
            v1 = act_tile("S6", "v1")
            o0 = act_tile("S1", "o0", FP8)  # o/8, after imgp's last read
            o1 = act_tile("S2", "o1", FP8)
            pca_cm.__exit__(None, None, None)  # free x1's 16KB for pqk
            sid4, _ = nc.enter_named_scope("P4", False)
            pqk_cm, pqk = open_pool(name="pqk", bufs=1)

            def qkv8(wt, act8, bname, mt, dst_t, dst_mt=None, scale=1.0,
                     dve=False):
                for ch in range(CH):
                    ps = pmain.tile([P, CHS], F32, tag="mm", name="ps_qkv")
                    mm_dr(ps, wt, act8, KT_H, ch)
                    bias = bias_sb[bname][:, mt:mt + 1] if bname else zero_col[:, :]
                    dst = (dst_t[:, _chsl(ch)] if dst_mt is None
                           else dst_t[:, dst_mt, _chsl(ch)])
                    if dve:
                        # DVE eviction: ACT is the tighter engine in P4
                        nc.vector.tensor_scalar(dst, ps[:], scale, bias,
                                                op0=ALU.mult, op1=ALU.add)
                    else:
                        nc.scalar.activation(dst, ps[:], AF.Identity,
                                             bias=bias, scale=scale)

            hm2 = hmask_sb[:, :]    # [128, 2] local-head one-hot
            AB = float(SA / SB)  # hmaskT_sb[0:2] is the [2,128] broadcast mask

            def gemm_part(mt, wts):
                """GEMMs + the DVE muls that feed the score chain."""
                wq_t, wv_t, wk_t, wqb_t, wvb_t = wts
                q0t = pqk.tile([P, R], BF16, tag="q0t", bufs=2)
                q1t = pqk.tile([P, R], BF16, tag="q1t", bufs=2)
                kdt = pqk.tile([P, R], BF16, tag="kdt")
                qkv8(wq_t, c0_f8, "sbq", mt, q0t)
                qkv8(wk_t, cd_f8, None, mt, kdt)  # k0-k1; bias cancels
                qkv8(wqb_t, c1_f8, "sbq", mt, q1t, scale=AB)
                nc.vector.tensor_mul(out=q0t[:], in0=q0t[:], in1=kdt[:])
                nc.vector.tensor_mul(out=q1t[:], in0=q1t[:], in1=kdt[:])
                qkv8(wv_t, c0_f8, "sbv8", mt, v0, dst_mt=mt, scale=1.0 / 8.0)
                qkv8(wvb_t, c1_f8, "sbv8", mt, v1, dst_mt=mt, scale=1.0 / SB)
                return mt, q0t, q1t

            def tail_part(st):
                """Score matmuls + attention combine for a PREVIOUS mt:
                emitted one iteration late so the cross-engine latency
                chain (evict->mul->dmm->sigmoid->ab) hides under the next
                mt's GEMM stream instead of stalling the PE."""
                mt, m0, m1 = st
                a_ts = {}
                for ch in range(CH):
                    for m_t, nm in ((m0, "A"), (m1, "B")):
                        dmm = paux.tile([2, CHS], F32, tag="Sb", name=f"dmm{nm}")
                        nc.tensor.matmul(dmm[:], lhsT=hm2, rhs=m_t[:, _chsl(ch)],
                                         start=True, stop=True)
                        a_t = spool.tile([2, CHS], BF16, tag="small", name=f"a{nm}")
                        nc.scalar.activation(a_t[:], dmm[:], AF.Sigmoid,
                                             bias=zero_col[0:2, :],
                                             scale=float(1.0 / np.sqrt(HD)))
                        a_ts[(ch, nm)] = a_t
                for ch in range(CH):
                    diff = tpool.tile([P, CHS], BF16, tag="tmp", name="att_diff")
                    nc.vector.tensor_sub(out=diff[:], in0=v0[:, mt, _chsl(ch)],
                                         in1=v1[:, mt, _chsl(ch)])
                    for o_t, nm in ((o0, "A"), (o1, "B")):
                        ab = paux.tile([P, CHS], F32, tag="Qb", name=f"ab{nm}")
                        nc.tensor.matmul(ab[:], lhsT=hmaskT_sb[0:2, :],
                                         rhs=a_ts[(ch, nm)][:],
                                         start=True, stop=True)
                        t_t = tpool.tile([P, CHS], BF16, tag="tmp", name=f"att_t{nm}")
                        nc.vector.tensor_mul(out=t_t[:], in0=diff[:], in1=ab[:])
                        nc.vector.tensor_add(out=o_t[:, mt, _chsl(ch)], in0=t_t[:],
                                             in1=v1[:, mt, _chsl(ch)])

            wq_next = None
            pend = None
            for mt in range(KT_H):
                if mt == 0:
                    wq0, wk0, wqb0 = qkv_pre
                    wts = (wq0,
                           load_w8(wvT, KT_H, 0, "wv"),
                           wk0, wqb0,
                           load_w8(wvbT, KT_H, 0, "wvb"))
                else:
                    wts = (wq_next,
                           load_w8(wvT, KT_H, mt, "wv"),
                           load_w8(wkT, KT_H, mt, "wk"),
                           load_w8(wqbT, KT_H, mt, "wqb"),
                           load_w8(wvbT, KT_H, mt, "wvb"))
                st = gemm_part(mt, wts)
                if mt + 1 < KT_H:
                    wq_next = load_w8(wqT, KT_H, mt + 1, "wq")
                if pend is not None:
                    tail_part(pend)
                pend = st
            tail_part(pend)
            # prefetch P5's first wo tile into the w_h ring
            wo_pre = load_w8(woT, KT_H, 0, "wo")
            nc.leave_named_scope("P4", sid4, False)

            # prefetch the first two FFN w1 tiles into the big-weight slots
            w1_pre = []
            for mt in range(2):
                t = wpool.tile([P, KT_F, P], FP8, tag="w", name=f"w1_pre{mt}")
                nc.sync.dma_start(t[:, :KT_H, :], w1aT[:, mt])
                w1_pre.append(t)

            # ===== P5: SA out-proj (fp8 DR) + residual + LN3 =====
            # Residual folded into PSUM via an identity matmul; ACT evicts.
            # LN3's bf16 output is stored as r + ffn_b2 (n3bf = n3_b + fb2)
            # so the FFN w2 evict needs no extra bias op.
            r0 = act_tile("S1", "r0")    # r0 + fb2 (bf16); reuses o0 slot
            r1 = act_tile("S2", "r1")
            r0_f8 = act_tile("C8a", "r0_f8", FP8, pad16=False)
            r1_f8 = act_tile("C8b", "r1_f8", FP8, pad16=False)
            imgp2 = act_tile("S3", "imgp2")
            txtp2 = act_tile("S4", "txtp2")
            pqk_cm.__exit__(None, None, None)  # free 10KB for psa
            sid5, _ = nc.enter_named_scope("P5", False)
            psa_cm, psa = open_pool(name="psa", bufs=1)
            y0 = sp_tile("y0")
            y1 = psa.tile([P, KT_H, R], BF16, tag="y1", name="y1")
            y2a = act_tile("S5", "y2a")  # y0^2; recycles v0 slot
            y2b = act_tile("S6", "y2b")

            # y0's whole pipeline runs FIRST so its stats/norm chain (the
            # FFN pos0 critical path) starts ~17us earlier; y1's GEMMs then
            # cover the y0 norm DVE work. wo tiles are loaded twice (+1MB).
            def wo_pass(o_t, res, dst, y2, first_pass):
                for mt in range(KT_H):
                    if first_pass and mt == 0:
                        wt = wo_pre
                    else:
                        wt = load_w8(woT, KT_H, mt, "wo")
                    for ch in range(CH):
                        ps = pmain.tile([P, CHS], F32, tag="mm", name="ps_wo")
                        mm_dr(ps, wt, o_t, KT_H, ch, stop=False)
                        nc.tensor.matmul(ps[:], lhsT=ident_sb[:],
                                         rhs=res[:, mt, _chsl(ch)],
                                         start=False, stop=True)
                        nc.scalar.activation(dst[:, mt, _chsl(ch)], ps[:],
                                             AF.Identity,
                                             bias=bias_sb["sbo"][:, mt:mt + 1],
                                             scale=1.0)
                    for ch in range(CH):
                        nc.vector.tensor_mul(out=y2[:, mt, _chsl(ch)],
                                             in0=dst[:, mt, _chsl(ch)],
                                             in1=dst[:, mt, _chsl(ch)])

            wo_pass(o0, c0, y0, y2a, True)
            for ch in range(CH):
                sty = stats_mms(y0, y2a, ch)
                ln_norm_raw(y0, sty, ch, r0, r0_f8)
            wo_pass(o1, c1, y1, y2b, False)
            for ch in range(CH):
                ln_gain_inplace(r0, ch, "n3gf", "n3bf")
            sty1 = [stats_mms(y1, y2b, ch) for ch in range(CH)]
            for ch in range(CH):
                ln_norm_raw(y1, sty1[ch], ch, r1, r1_f8, f8_dve=True)
                ln_gain_inplace(r1, ch, "n3gf", "n3bf")

            # ===== P6/P7: FFN both positions (fp8 DR); pooled accumulation
            # SAME pool scope as P5: a separate scope would barrier the FFN
            # start on the y1-stats PSUM retirement (9.5us measured).
            # mt-outer / ch-inner: each weight tile loaded once per position.
            pooled = sp_tile("pooled")  # bf16; pos1 fuses the final combine
            sid67, _ = nc.enter_named_scope("P67", False)
            gate_done = False
            for pos, (r_f8, r_p, w1d, w2d, first) in enumerate([
                    (r0_f8, r0, w1aT, w2aT, True),
                    (r1_f8, r1, w1bT, w2bT, False)]):
                if not first and not gate_done:
                    # g_sb <- gate*(imgp-txtp) + txtp in place; runs on DVE
                    # during pos0's PE work so the pos1 chain is 3 ops + DMA
                    gate_done = True
                    for gmt in range(KT_H):
                        for gch in range(CH):
                            gsl = _chsl(gch)
                            gd = tpool.tile([P, CHS], BF16, tag="tmp", name="gd")
                            nc.vector.tensor_sub(out=gd[:],
                                                 in0=imgp2[:, gmt, gsl],
                                                 in1=txtp2[:, gmt, gsl])
                            gt = tpool.tile([P, CHS], BF16, tag="tmp", name="gt")
                            nc.vector.tensor_mul(out=gt[:],
                                                 in0=g_sb[:, gmt, gsl], in1=gd[:])
                            nc.vector.tensor_add(out=g_sb[:, gmt, gsl],
                                                 in0=gt[:], in1=txtp2[:, gmt, gsl])
                            # fold the gate term into pooled here (DVE has
                            # slack under pos1's w1 GEMMs): shortens the
                            # per-(mt,ch) output chain to 2 ops
                            nc.vector.tensor_add(out=pooled[:, gmt, gsl],
                                                 in0=pooled[:, gmt, gsl],
                                                 in1=g_sb[:, gmt, gsl])
                gelu_scale = 1.0 if first else float(SA / SB)
                ev_scale = (1.0 / SA) if first else (1.0 / SB)
                # hidden [128, 16, 1024] fp8 x2 in the freed y2a/y2b slots
                h_a = acts.tile([P, KT_F // 2, R], FP8, tag="S5",
                                name=f"h_a{pos}")
                h_b = acts.tile([P, KT_F // 2, R], FP8, tag="S6",
                                name=f"h_b{pos}")
                if first:
                    # start with ch0-only chains: r0_f8[ch1] finishes on DVE
                    # a few us after ch0, so ch1 work is deferred past it
                    # (mt0/1 sit in the big w slots, so all 8 fit the ring)
                    NCH0 = 8
                    w1_order = ([(mt, 0) for mt in range(NCH0)]
                                + [(mt, 1) for mt in range(NCH0)]
                                + [(mt, ch) for mt in range(NCH0, KT_F)
                                   for ch in range(CH)])
                else:
                    w1_order = [(mt, ch) for mt in range(KT_F)
                                for ch in range(CH)]
                w1_tiles = {}
                for mt, ch in w1_order:
                    if mt not in w1_tiles:
                        if first and mt < 2:
                            w1_tiles[mt] = w1_pre[mt]
                        else:
                            w1_tiles[mt] = load_w8(w1d, KT_H, mt, "w1")
                    if first and ch == 0:
                        # reload the P8 gate operands (4MB) here, past the
                        # FFN head, so they don't contend with the w1 stream
                        if mt == 10:
                            nc.sync.dma_start(imgp2[:], imgp_d[:, :, :])
                        elif mt == 20:
                            nc.sync.dma_start(txtp2[:], txtp_d[:, :, :])
                    wt = w1_tiles[mt]
                    hdst = h_a if mt < KT_F // 2 else h_b
                    ps = pmain.tile([P, CHS], F32, tag="mm", name="ps_f1")
                    mm_dr(ps, wt, r_f8, KT_H, ch)
                    nc.scalar.activation(
                        hdst[:, mt % (KT_F // 2), _chsl(ch)], ps[:],
                        AF.Gelu, bias=fb1_sb[:, mt:mt + 1], scale=gelu_scale)
                for mt in range(KT_H):
                    wt = load_w8(w2d, KT_F, mt, "w2")
                    for ch in range(CH):
                        ps = pmain.tile([P, CHS], F32, tag="mm", name="ps_f2")
                        for k in range(KT_F // 4):
                            nc.tensor.matmul(
                                ps[:], lhsT=wt[:, 2 * k:2 * k + 2, :],
                                rhs=h_a[:, 2 * k:2 * k + 2, _chsl(ch)],
                                start=(k == 0), stop=False, perf_mode=DR)
                        for k in range(KT_F // 4):
                            nc.tensor.matmul(
                                ps[:],
                                lhsT=wt[:, KT_F // 2 + 2 * k:KT_F // 2 + 2 * k + 2, :],
                                rhs=h_b[:, 2 * k:2 * k + 2, _chsl(ch)],
                                start=False, stop=(k == KT_F // 4 - 1),
                                perf_mode=DR)
                        if first:
                            # r0 holds 0.5*(r+fb2): pooled = 0.5*p0
                            nc.vector.scalar_tensor_tensor(
                                pooled[:, mt, _chsl(ch)], ps[:], 0.5 * ev_scale,
                                r_p[:, mt, _chsl(ch)], op0=ALU.mult, op1=ALU.add)
                        else:
                            # final combine (gate pre-added into pooled):
                            # out = [0.5*p1] + [0.5*p0 + gate-term]
                            tmp = tpool.tile([P, CHS], F32, tag="tmp", name="ffn_tmp")
                            nc.vector.scalar_tensor_tensor(
                                tmp[:], ps[:], 0.5 * ev_scale,
                                r_p[:, mt, _chsl(ch)], op0=ALU.mult, op1=ALU.add)
                            fin = tpool.tile([P, CHS], F32, tag="tmp", name="gfin")
                            nc.vector.tensor_add(out=fin[:], in0=tmp[:],
                                                 in1=pooled[:, mt, _chsl(ch)])
                            nc.sync.dma_start(outT[mt * P:(mt + 1) * P, _chsl(ch)],
                                              fin[:])
            nc.leave_named_scope("P67", sid67, False)
            nc.leave_named_scope("P5", sid5, False)
            psa_cm.__exit__(None, None, None)

        lnp_cm.__exit__(None, None, None)
        acts_cm.__exit__(None, None, None)
        spool_cm.__exit__(None, None, None)
        tpool_cm.__exit__(None, None, None)
        wpool_cm.__exit__(None, None, None)
        const_cm.__exit__(None, None, None)

    nc.compile()
    return nc


def host_prep(inputs):
    """Host-side preprocessing: merge CA weights, transpose, cast, shard."""
    f = {k: np.asarray(v, dtype=np.float32) for k, v in inputs.items()}

    def bf(x):
        return np.ascontiguousarray(x).astype(np_bf16)

    def tile4(wT, np_dt):
        """[K, M] (already transposed) -> [P, MT, KT, P] pre-tiled layout."""
        K, M = wT.shape
        kt, mtn = K // P, M // P
        w4 = wT.reshape(kt, P, mtn, P).transpose(1, 2, 0, 3)
        return np.ascontiguousarray(w4).astype(np_dt)

    def q8t(w, s):
        """quantize w.T at scale s, pre-tiled."""
        return tile4(np.asarray(w, np.float32).T * s, np_fp8)

    def bft(w):
        return tile4(np.asarray(w, np.float32).T, np_bf16)

    def bias128(x, kt):
        return np.ascontiguousarray(np.asarray(x, np.float32).reshape(kt, P).T)

    ca_wv = np.split(f["ca_wqkv"], 3, axis=0)[2]
    ca_bv = f["ca_bqkv"][2 * H:]
    w_vo = f["ca_wo"] @ ca_wv
    b_vo = f["ca_wo"] @ ca_bv + f["ca_bo"]

    sa_wq, sa_wk, sa_wv = np.split(f["sa_wqkv"], 3, axis=0)
    sa_bq, sa_bk, sa_bv = np.split(f["sa_bqkv"], 3)

    gwi = f["gate_w"][:, :H]
    gwt = f["gate_w"][:, H:]

    # LN3 gain/bias fold for the FFN first layer (see kernel docstring)
    w1f = f["ffn_w1"] * f["n3_g"][None, :]
    fb1f = f["ffn_b1"] + f["ffn_w1"] @ f["n3_b"]

    lh = np.arange(P) // HD  # local head index within a 128-feature tile
    hmask = np.ascontiguousarray((lh[:, None] == np.arange(2)[None, :]).astype(np_bf16))
    hmaskT = np.zeros((34, P), np_bf16)  # copies at rows 0-1 and 32-33
    hmaskT[0:2] = hmask.T
    hmaskT[32:34] = hmask.T
    hmaskT = np.ascontiguousarray(hmaskT)

    shared = {
        "wiT": bf(f["Wi"].T), "wtT": bf(f["Wt"].T),
        "wvoT": bft(w_vo),
        "wqT": q8t(sa_wq, SA), "wqbT": q8t(sa_wq, SB),
        "wkT": q8t(sa_wk, SA),
        "wvT": q8t(sa_wv, SA), "wvbT": q8t(sa_wv, SB),
        "woT": q8t(f["sa_wo"], SA),
        # LN3 gain/bias folded into w1 (input-column scale) and its bias:
        # gelu(u@(w1*g).T + b1 + w1@b) == gelu((u*g+b)@w1.T + b1)
        "w1aT": q8t(w1f, SA), "w1bT": q8t(w1f, SB),
        "w2aT": q8t(f["ffn_w2"], SA), "w2bT": q8t(f["ffn_w2"], SB),
        "gwiT": bft(gwi), "gwtT": bft(gwt),
        "ident128": np.ascontiguousarray(np.eye(P, dtype=np_bf16)),
        "bias_all": np.concatenate([
            bias128(f["bi"], KT_H), bias128(f["bt"], KT_H), bias128(b_vo, KT_H),
            bias128(sa_bq, KT_H), bias128(sa_bv / 8.0, KT_H),
            bias128(f["sa_bo"], KT_H), bias128(f["ffn_b2"], KT_H),
            bias128(f["gate_b"], KT_H),
            bias128(f["n1_g"], KT_H), bias128(f["n1_b"], KT_H),
            bias128(f["n2_g"], KT_H), bias128(f["n2_b"], KT_H),
            bias128(0.5 * f["n3_g"], KT_H),
            bias128(0.5 * (f["n3_b"] + f["ffn_b2"]), KT_H),
            bias128(-f["ffn_b2"] / 8.0, KT_H),
            bias128(fb1f, KT_F),
        ], axis=1),
        "hmask": np.ascontiguousarray(hmask), "hmaskT": hmaskT,
    }

    xiT = f["image_features"].T.astype(np_bf16)  # [IMG_D, B]
    xtT = f["text_features"].T.astype(np_bf16)
    in_maps = []
    for c in range(N_CORES):
        m = dict(shared)
        m["xiT"] = np.ascontiguousarray(xiT[:, c * R:(c + 1) * R])
        m["xtT"] = np.ascontiguousarray(xtT[:, c * R:(c + 1) * R])
        in_maps.append(m)
    return in_maps


_NC_CACHE = None


def kernel(**inputs) -> np.ndarray:
    global _NC_CACHE
    if _NC_CACHE is None:
        _NC_CACHE = build()
    nc = _NC_CACHE
    in_maps = host_prep(inputs)
    res = run_bass_kernel_spmd(nc, in_maps, core_ids=list(range(N_CORES)))
    out = np.empty((B, H), np.float32)
    for c in range(N_CORES):
        out[c * R:(c + 1) * R, :] = res.results[c]["outT"].T
    return out


if __name__ == "__main__":
    nc = build()
    print("built OK")
